# revision 2
# baseline (speedup 1.0000x reference)
# Lagrangian-NN qddot kernel for TRN2 (8 NeuronCores, data-parallel over batch).
#
# Math: scalar L(q,qdot) = MLP(24->256x4->1, softplus). Per sample:
#   M = d2L/dqdot2 + 0.01 I ; C = d2L/dqdot dq ; qddot = M^-1 (dL/dq - C qdot).
# Batched fwd+bwd gives grad; 12 qdot-direction forward-over-reverse tangents give
# H[:,12:] whose symmetry supplies both M and the Coriolis contraction; per-sample
# 12x12 solve by batched no-pivot Gauss-Jordan (M ~ 0.01*I, cond ~1.06).
# softplus/sigmoid composed from {abs,exp,ln,relu} (single ACT table set):
#   Z = relu(x) + ln(exp(-|x|)+1) ; S = exp(x - Z).
import sys
import numpy as np

for p in ("/opt/trn_rl_repo", "/root/.axon_site/_ro/trn_rl_repo"):
    if p not in sys.path:
        sys.path.insert(0, p)

import concourse.bass as bass
import concourse.mybir as mybir
import concourse.tile as tile
from concourse import bacc
from concourse.bass_utils import run_bass_kernel_spmd

F32 = mybir.dt.float32
F32R = mybir.dt.float32r
AF = mybir.ActivationFunctionType
ALU = mybir.AluOpType

B, ND, H, NC = 8192, 12, 256, 8
N = B // NC
IN = 2 * ND
T = 64
NT = N // T
NG = N // 128
FD = ND * T
CH = 512
KT = H // 128

_cache = {}


def build_kernel():
    nc = bacc.Bacc("TRN2", target_bir_lowering=False)
    dq = nc.dram_tensor("q", (N, ND), F32, kind="ExternalInput")
    dqd = nc.dram_tensor("qdot", (N, ND), F32, kind="ExternalInput")
    dWT = [nc.dram_tensor(f"WT{l}", s, F32, kind="ExternalInput")
           for l, s in enumerate([(IN, H), (H, H), (H, H), (H, H)])]
    dWn = {l: nc.dram_tensor(f"Wn{l}", (H, H), F32, kind="ExternalInput") for l in (1, 2, 3)}
    dW0 = nc.dram_tensor("W0n", (H, IN), F32, kind="ExternalInput")
    dbs = [nc.dram_tensor(f"b{l}", (H, 1), F32, kind="ExternalInput") for l in range(4)]
    dw4 = nc.dram_tensor("w4", (H, 1), F32, kind="ExternalInput")
    dide = nc.dram_tensor("ident", (128, 128), F32, kind="ExternalInput")
    dout = nc.dram_tensor("qdd", (N, ND), F32, kind="ExternalOutput")

    with tile.TileContext(nc) as tc:
        with tc.tile_pool(name="wp", bufs=1) as wp, \
             tc.tile_pool(name="acts", bufs=1) as actp, \
             tc.tile_pool(name="tang", bufs=1) as tgp, \
             tc.tile_pool(name="scr", bufs=1) as scr, \
             tc.tile_pool(name="psB", bufs=2, space="PSUM") as psB, \
             tc.tile_pool(name="psT", bufs=2, space="PSUM") as psT:

            ident = wp.tile([128, 128], F32)
            nc.sync.dma_start(ident[:], dide[:])

            def load_round(dram, P, Fr, tag):
                parts = []
                for ki, p0 in enumerate(range(0, P, 128)):
                    pe = min(P, p0 + 128)
                    raw = scr.tile([pe - p0, Fr], F32, tag="wraw")
                    nc.sync.dma_start(raw[:], dram[p0:pe, :])
                    r = wp.tile([pe - p0, Fr], F32R, tag=f"{tag}_{ki}")
                    nc.scalar.activation(r[:], raw[:], AF.Copy)
                    parts.append(r)
                return parts

            WT_r = [load_round(dWT[l], (IN if l == 0 else H), H, f"WT{l}") for l in range(4)]
            Wn_r = {l: load_round(dWn[l], H, H, f"Wn{l}") for l in (1, 2, 3)}
            W0_r = load_round(dW0, H, IN, "W0n")
            W0q = []
            for ki in range(KT):
                t = wp.tile([128, ND], F32, tag=f"W0q{ki}")
                nc.sync.dma_start(t[:], dW0[ki * 128:(ki + 1) * 128, ND:])
                W0q.append(t)
            bs = []
            for l in range(4):
                ps_ = []
                for ki in range(KT):
                    t = wp.tile([128, 1], F32, tag=f"b{l}_{ki}")
                    nc.sync.dma_start(t[:], dbs[l][ki * 128:(ki + 1) * 128, :])
                    ps_.append(t)
                bs.append(ps_)
            w4t = []
            for ki in range(KT):
                t = wp.tile([128, 1], F32, tag=f"w4_{ki}")
                nc.sync.dma_start(t[:], dw4[ki * 128:(ki + 1) * 128, :])
                w4t.append(t)

            xts = []
            XT = actp.tile([IN, N], F32R)
            for g in range(NG):
                xt = actp.tile([128, IN], F32, tag=f"xt{g}")
                nc.sync.dma_start(xt[:, 0:ND], dq[g * 128:(g + 1) * 128, :])
                nc.sync.dma_start(xt[:, ND:], dqd[g * 128:(g + 1) * 128, :])
                xts.append(xt)
                pt = psT.tile([IN, 128], F32, tag="ptx")
                nc.tensor.transpose(pt[:], xt[:], ident[:])
                nc.scalar.activation(XT[:, g * 128:(g + 1) * 128], pt[:], AF.Copy)

            def mm(psum_ap, lhsT_parts, rhs_parts, Fr):
                nk = len(lhsT_parts)
                for c0 in range(0, Fr, CH):
                    ce = min(Fr, c0 + CH)
                    for ki in range(nk):
                        nc.tensor.matmul(psum_ap[:, c0:ce], lhsT_parts[ki],
                                         rhs_parts[ki][:, c0:ce],
                                         start=(ki == 0), stop=(ki == nk - 1))

            def fwd_layer(rhs_parts, Wparts, K, lidx):
                Zs, Ss = [], []
                nk = (K + 127) // 128
                for ot in range(KT):
                    ps = psB.tile([128, 1024], F32, tag="big")
                    lts = [Wparts[k][:, ot * 128:(ot + 1) * 128] for k in range(nk)]
                    mm(ps[:, 0:N], lts, rhs_parts, N)
                    Ap = scr.tile([128, N], F32, tag="Ap")
                    nc.scalar.activation(Ap[:], ps[0:128, 0:N], AF.Identity,
                                         bias=bs[lidx][ot][:])
                    ab = scr.tile([128, N], F32, tag="ab")
                    nc.scalar.activation(ab[:], Ap[:], AF.Abs)
                    ex = scr.tile([128, N], F32, tag="ex")
                    nc.scalar.activation(ex[:], ab[:], AF.Exp, scale=-1.0)
                    ln = scr.tile([128, N], F32, tag="ln")
                    nc.scalar.activation(ln[:], ex[:], AF.Ln, bias=1.0)
                    rl = scr.tile([128, N], F32, tag="rl")
                    nc.scalar.activation(rl[:], Ap[:], AF.Relu)
                    Z = actp.tile([128, N], F32R, tag=f"Zf{lidx % 2}_{ot}")
                    nc.vector.tensor_add(Z[:], rl[:], ln[:])
                    d = scr.tile([128, N], F32, tag="d")
                    nc.vector.tensor_sub(d[:], Ap[:], Z[:].bitcast(F32))
                    S = actp.tile([128, N], F32, tag=f"S{lidx}_{ot}")
                    nc.scalar.activation(S[:], d[:], AF.Exp)
                    Zs.append(Z)
                    Ss.append(S)
                return Zs, Ss

            Z1, S1 = fwd_layer([XT[:]], WT_r[0], IN, 0)
            Z2, S2 = fwd_layer([z[:] for z in Z1], WT_r[1], H, 1)
            Z3, S3 = fwd_layer([z[:] for z in Z2], WT_r[2], H, 2)
            _, S4 = fwd_layer([z[:] for z in Z3], WT_r[3], H, 3)

            D4, c4 = [], []
            for ot in range(KT):
                D = actp.tile([128, N], F32R, tag=f"Dr0_{ot}")
                nc.vector.tensor_scalar_mul(D[:], S4[ot][:], w4t[ot][:])
                D4.append(D)
                t1 = scr.tile([128, N], F32, tag="c4t")
                nc.vector.tensor_mul(t1[:], D[:].bitcast(F32), S4[ot][:])
                c = actp.tile([128, N], F32, tag=f"c4_{ot}")
                nc.vector.tensor_sub(c[:], D[:].bitcast(F32), t1[:])
                c4.append(c)

            def bwd_layer(Dup, Wparts, Sl, lidx, want_F):
                Ds, Fs = [], []
                for ot in range(KT):
                    ps = psB.tile([128, 1024], F32, tag="big")
                    lts = [Wparts[k][:, ot * 128:(ot + 1) * 128] for k in range(KT)]
                    mm(ps[:, 0:N], lts, [d[:] for d in Dup], N)
                    D = actp.tile([128, N], F32R, tag=f"Dr{lidx % 2}_{ot}")
                    nc.vector.tensor_mul(D[:], Sl[ot][:], ps[0:128, 0:N])
                    Ds.append(D)
                    if want_F:
                        F = actp.tile([128, N], F32, tag=f"F{lidx}_{ot}")
                        nc.vector.tensor_sub(F[:], ps[0:128, 0:N], D[:].bitcast(F32))
                        Fs.append(F)
                return Ds, Fs

            D3, F3 = bwd_layer(D4, Wn_r[3], S3, 3, True)
            D2, F2 = bwd_layer(D3, Wn_r[2], S2, 2, True)
            D1, _ = bwd_layer(D2, Wn_r[1], S1, 1, False)
            E1 = []
            for ot in range(KT):
                t1 = scr.tile([128, N], F32, tag="e1t")
                nc.vector.tensor_mul(t1[:], D1[ot][:].bitcast(F32), S1[ot][:])
                E = actp.tile([128, N], F32, tag=f"E1_{ot}")
                nc.vector.tensor_sub(E[:], D1[ot][:].bitcast(F32), t1[:])
                E1.append(E)

            Gps = psB.tile([128, 1024], F32, tag="big")
            mm(Gps[0:IN, 0:N], [W0_r[k][:] for k in range(KT)], [d[:] for d in D1], N)
            G = actp.tile([IN, N], F32)
            nc.scalar.activation(G[:], Gps[0:IN, 0:N], AF.Copy)

            aug = actp.tile([128, 13 * ND * NG], F32)
            aug4 = aug[:].rearrange("p (i j g) -> p i j g", i=ND, j=13, g=NG)

            for g in range(NG):
                ptg = psT.tile([128, IN], F32, tag="ptx")
                nc.tensor.transpose(ptg[0:128, 0:ND], G[0:ND, g * 128:(g + 1) * 128],
                                    ident[0:ND, 0:ND])
                nc.scalar.activation(aug4[:, :, 12, g], ptg[0:128, 0:ND], AF.Copy)

            Hc128 = CHq = None
            for b in range(NT):
                g, off = b // 2, (b % 2) * 64
                sl = slice(b * T, (b + 1) * T)

                def bca(Sten, ot):
                    return Sten[ot][:, sl].unsqueeze(1).broadcast_to((128, ND, T))

                def t3d(ps):
                    return ps[0:128, 0:FD].rearrange("p (d t) -> p d t", d=ND)

                Zd1 = []
                for ot in range(KT):
                    z = tgp.tile([128, ND, T], F32R, tag=f"ZdA_{ot}")
                    wq = W0q[ot][:].unsqueeze(2).broadcast_to((128, ND, T))
                    nc.vector.tensor_tensor(z[:], bca(S1, ot), wq, ALU.mult)
                    Zd1.append(z)

                def tang_fwd(Zin, Wparts, Sl, ztag):
                    outs = []
                    for ot in range(KT):
                        ps = psB.tile([128, 1024], F32, tag="big")
                        lts = [Wparts[k][:, ot * 128:(ot + 1) * 128] for k in range(KT)]
                        mm(ps[:, 0:FD], lts,
                           [z[:].rearrange("p d t -> p (d t)") for z in Zin], FD)
                        z = tgp.tile([128, ND, T], F32R, tag=f"Zd{ztag}_{ot}")
                        nc.vector.tensor_tensor(z[:], bca(Sl, ot), t3d(ps), ALU.mult)
                        outs.append(z)
                    return outs

                Zd2 = tang_fwd(Zd1, WT_r[1], S2, "B")
                Zd3 = tang_fwd(Zd2, WT_r[2], S3, "C")

                Dd4 = []
                for ot in range(KT):
                    ps = psB.tile([128, 1024], F32, tag="big")
                    lts = [WT_r[3][k][:, ot * 128:(ot + 1) * 128] for k in range(KT)]
                    mm(ps[:, 0:FD], lts,
                       [z[:].rearrange("p d t -> p (d t)") for z in Zd3], FD)
                    dd = tgp.tile([128, ND, T], F32R, tag=f"DdA_{ot}")
                    nc.vector.tensor_tensor(dd[:], bca(c4, ot), t3d(ps), ALU.mult)
                    Dd4.append(dd)

                def tang_bwd(Ddup, Wparts, Sl, Fl, Zdl, dtag):
                    outs = []
                    for ot in range(KT):
                        ps = psB.tile([128, 1024], F32, tag="big")
                        lts = [Wparts[k][:, ot * 128:(ot + 1) * 128] for k in range(KT)]
                        mm(ps[:, 0:FD], lts,
                           [d[:].rearrange("p d t -> p (d t)") for d in Ddup], FD)
                        tb = scr.tile([128, ND, T], F32, tag="tB")
                        nc.vector.tensor_tensor(tb[:], bca(Sl, ot), t3d(ps), ALU.mult)
                        ta = scr.tile([128, ND, T], F32, tag="tA2")
                        nc.vector.tensor_tensor(ta[:], bca(Fl, ot),
                                                Zdl[ot][:].bitcast(F32), ALU.mult)
                        dd = tgp.tile([128, ND, T], F32R, tag=f"Dd{dtag}_{ot}")
                        nc.gpsimd.tensor_add(dd[:].rearrange("p d t -> p (d t)"),
                                             ta[:].rearrange("p d t -> p (d t)"),
                                             tb[:].rearrange("p d t -> p (d t)"))
                        outs.append(dd)
                    return outs

                Dd3 = tang_bwd(Dd4, Wn_r[3], S3, F3, Zd3, "B")
                Dd2 = tang_bwd(Dd3, Wn_r[2], S2, F2, Zd2, "A")

                Dd1 = []
                for ot in range(KT):
                    ps = psB.tile([128, 1024], F32, tag="big")
                    lts = [Wn_r[1][k][:, ot * 128:(ot + 1) * 128] for k in range(KT)]
                    mm(ps[:, 0:FD], lts,
                       [d[:].rearrange("p d t -> p (d t)") for d in Dd2], FD)
                    tb = scr.tile([128, ND, T], F32, tag="tB")
                    nc.vector.tensor_tensor(tb[:], bca(S1, ot), t3d(ps), ALU.mult)
                    ta = scr.tile([128, ND, T], F32, tag="tA2")
                    wq = W0q[ot][:].unsqueeze(2).broadcast_to((128, ND, T))
                    nc.vector.tensor_tensor(ta[:], bca(E1, ot), wq, ALU.mult)
                    dd = tgp.tile([128, ND, T], F32R, tag=f"DdB_{ot}")
                    nc.gpsimd.tensor_add(dd[:].rearrange("p d t -> p (d t)"),
                                         ta[:].rearrange("p d t -> p (d t)"),
                                         tb[:].rearrange("p d t -> p (d t)"))
                    Dd1.append(dd)

                psH = psB.tile([128, 1024], F32, tag="big")
                mm(psH[0:IN, 0:FD], [W0_r[k][:] for k in range(KT)],
                   [d[:].rearrange("p d t -> p (d t)") for d in Dd1], FD)
                if off == 0:
                    Hc128 = scr.tile([IN, ND, 128], F32, tag="Hc")
                    CHq = scr.tile([128, ND * ND], F32, tag="CHq")
                nc.scalar.activation(Hc128[:, :, off:off + T],
                                     psH[0:IN, 0:FD].rearrange("p (d t) -> p d t", d=ND),
                                     AF.Copy)

                if off == 64:
                    for d in range(ND):
                        pt = psT.tile([128, IN], F32, tag="ptH")
                        nc.tensor.transpose(pt[:], Hc128[:, d, :], ident[0:IN, 0:IN])
                        nc.scalar.activation(CHq[:, d * ND:(d + 1) * ND],
                                             pt[:, 0:ND], AF.Copy)
                        nc.scalar.activation(aug4[:, :, d, g], pt[:, ND:IN], AF.Copy)
                    prod = scr.tile([128, ND, ND], F32, tag="prod")
                    qdv = xts[g][:, ND:IN].unsqueeze(1).broadcast_to((128, ND, ND))
                    nc.vector.tensor_tensor(prod[:], CHq[:].rearrange("p (i j) -> p i j", j=ND),
                                            qdv, ALU.mult)
                    cor = scr.tile([128, ND], F32, tag="cor")
                    nc.vector.tensor_reduce(cor[:].unsqueeze(2), prod[:], op=ALU.add,
                                            axis=mybir.AxisListType.X)
                    nc.vector.tensor_sub(aug4[:, :, 12, g], aug4[:, :, 12, g], cor[:])

            for i in range(ND):
                nc.vector.tensor_scalar_add(aug4[:, i, i, :], aug4[:, i, i, :], 0.01)

            for k in range(ND):
                piv = aug4[:, k, k, :]
                rec = scr.tile([128, NG], F32, tag="rec")
                nc.vector.reciprocal(rec[:], piv)
                nw = 12 - k
                rk = aug4[:, k, k + 1:13, :]
                recb = rec[:].unsqueeze(1).broadcast_to((128, nw, NG))
                nc.vector.scalar_tensor_tensor(rk, rk, -1.0, recb, ALU.mult, ALU.mult)
                for i in range(ND):
                    if i == k:
                        continue
                    fb = aug4[:, i, k, :].unsqueeze(1).broadcast_to((128, nw, NG))
                    tmv = scr.tile([128, nw, NG], F32, tag="gjt")
                    nc.vector.tensor_tensor(tmv[:], rk, fb, ALU.mult)
                    nc.vector.tensor_add(aug4[:, i, k + 1:13, :], aug4[:, i, k + 1:13, :], tmv[:])

            for g in range(NG):
                xo = scr.tile([128, ND], F32, tag="xo")
                nc.vector.tensor_scalar_mul(xo[:], aug4[:, :, 12, g], -1.0)
                nc.sync.dma_start(dout[g * 128:(g + 1) * 128, :], xo[:])

    nc.compile()
    return nc


def kernel(**inputs):
    q = np.ascontiguousarray(inputs["q"], dtype=np.float32)
    qdot = np.ascontiguousarray(inputs["qdot"], dtype=np.float32)
    if "nc" not in _cache:
        _cache["nc"] = build_kernel()
    nc = _cache["nc"]
    base = {
        "WT0": np.ascontiguousarray(inputs["W0"].T).astype(np.float32),
        "WT1": np.ascontiguousarray(inputs["W1"].T).astype(np.float32),
        "WT2": np.ascontiguousarray(inputs["W2"].T).astype(np.float32),
        "WT3": np.ascontiguousarray(inputs["W3"].T).astype(np.float32),
        "Wn1": np.ascontiguousarray(inputs["W1"]).astype(np.float32),
        "Wn2": np.ascontiguousarray(inputs["W2"]).astype(np.float32),
        "Wn3": np.ascontiguousarray(inputs["W3"]).astype(np.float32),
        "W0n": np.ascontiguousarray(inputs["W0"]).astype(np.float32),
        "b0": inputs["b0"].reshape(H, 1).astype(np.float32),
        "b1": inputs["b1"].reshape(H, 1).astype(np.float32),
        "b2": inputs["b2"].reshape(H, 1).astype(np.float32),
        "b3": inputs["b3"].reshape(H, 1).astype(np.float32),
        "w4": np.ascontiguousarray(inputs["W4"].reshape(H, 1)).astype(np.float32),
        "ident": np.eye(128, dtype=np.float32),
    }
    in_maps = []
    for c in range(NC):
        m = dict(base)
        m["q"] = q[c * N:(c + 1) * N]
        m["qdot"] = qdot[c * N:(c + 1) * N]
        in_maps.append(m)
    res = run_bass_kernel_spmd(nc, in_maps, core_ids=list(range(NC)))
    _cache["res"] = res
    out = np.concatenate([res.results[c]["qdd"] for c in range(NC)], axis=0)
    return out.astype(np.float32)



# revision 17
# speedup vs baseline: 1.2733x; 1.2733x over previous
# Lagrangian-NN qddot kernel for TRN2 (8 NeuronCores, data-parallel over batch).
#
# Math: scalar L(q,qdot) = MLP(24->256x4->1, softplus). Per sample:
#   M = d2L/dqdot2 + 0.01 I ; C = d2L/dqdot dq ; qddot = M^-1 (dL/dq - C qdot).
# Batched fwd+bwd gives grad; 12 qdot-direction forward-over-reverse tangents give
# H[:,12:] whose symmetry supplies both M and the Coriolis contraction.
#
# v2: all matmuls bf16 (FastWeightLoad + 1024-wide moving operands); sigmoid/
# softplus chain in f32 on ACT {exp,ln} single act-table; tangent-phase PSUM
# evacuation on ACT (bf16 out) + DVE 2x products + 4x fused adds (stt);
# H layout transpose via XBAR DMA transpose (idle DMA engines); per-sample
# 12x12 solve by Jacobi iteration x <- 100(r - H22 x), contraction <= 0.035
# (M = 0.01(I+K), ||K|| small), 5 iters.
import sys
import numpy as np

for p in ("/opt/trn_rl_repo", "/root/.axon_site/_ro/trn_rl_repo"):
    if p not in sys.path:
        sys.path.insert(0, p)

import ml_dtypes
import concourse.bass as bass
import concourse.mybir as mybir
import concourse.tile as tile
from concourse import bacc
from concourse.bass_utils import run_bass_kernel_spmd

F32 = mybir.dt.float32
BF = mybir.dt.bfloat16
AF = mybir.ActivationFunctionType
ALU = mybir.AluOpType
bf16 = ml_dtypes.bfloat16

B, ND, H, NC = 8192, 12, 256, 8
N = B // NC          # 1024 samples per core
IN = 2 * ND          # 24
T = 64               # samples per tangent block
NT = N // T          # 16 blocks
NG = N // 128        # 8 groups of 128 samples
FD = ND * T          # 768 tangent free dim
KT = H // 128        # 2 partition chunks of the hidden dim
JAC_ITERS = 5

_cache = {}


def _patch_single_act_table():
    """Force every activation to resolve to natural_log_exp_and_others so a
    single ACT table load suffices (instead of ping-ponging exp/ln sets).
    Only the membership sets are modified; names/indices keep their original
    act_info.json order so emitted act_func_set_ids stay valid."""
    import concourse.bacc as bacc_mod
    from concourse.hw_specs import get_activation_tables as orig

    if getattr(bacc_mod, "_single_act_patch", False):
        return

    def patched(arch):
        t = orig(arch)
        return {k: (v if k == "natural_log_exp_and_others" else set())
                for k, v in t.items()}

    bacc_mod.get_activation_tables = patched
    bacc_mod._single_act_patch = True


def build_kernel():
    _patch_single_act_table()
    nc = bacc.Bacc("TRN2", target_bir_lowering=False)
    dq = nc.dram_tensor("q", (N, ND), F32, kind="ExternalInput")
    dqd = nc.dram_tensor("qdot", (N, ND), F32, kind="ExternalInput")
    dWT = [nc.dram_tensor(f"WT{l}", s, BF, kind="ExternalInput")
           for l, s in enumerate([(IN, H), (H, H), (H, H), (H, H)])]
    dWn = {l: nc.dram_tensor(f"Wn{l}", (H, H), BF, kind="ExternalInput") for l in (1, 2, 3)}
    dW0n = nc.dram_tensor("W0n", (H, IN), BF, kind="ExternalInput")
    dW0q = nc.dram_tensor("W0q", (H, ND), BF, kind="ExternalInput")
    dbs = [nc.dram_tensor(f"b{l}", (H, 1), F32, kind="ExternalInput") for l in range(4)]
    dw4 = nc.dram_tensor("w4", (H, 1), F32, kind="ExternalInput")
    dide = nc.dram_tensor("ident", (128, 128), F32, kind="ExternalInput")
    dhsc = nc.dram_tensor("hscratch", (NG, 32, ND * 128), BF, kind="Internal")
    dout = nc.dram_tensor("qdd", (N, ND), F32, kind="ExternalOutput")

    with tile.TileContext(nc) as tc:
        with tc.tile_pool(name="wp", bufs=1) as wp, \
             tc.tile_pool(name="ph1", bufs=1) as ph1, \
             tc.tile_pool(name="acts", bufs=1) as actp, \
             tc.tile_pool(name="evp", bufs=3) as evp, \
             tc.tile_pool(name="tang", bufs=2) as tgp, \
             tc.tile_pool(name="hcp", bufs=2) as hcp, \
             tc.tile_pool(name="tail", bufs=1) as tlp, \
             tc.tile_pool(name="xp", bufs=2) as xp, \
             tc.tile_pool(name="psB", bufs=3, space="PSUM") as psB, \
             tc.tile_pool(name="psT", bufs=1, space="PSUM") as psT:

            ident = wp.tile([128, 128], F32)
            nc.sync.dma_start(ident[:], dide[:])

            def load_bf(dram, P, Fr, tag):
                parts = []
                for ki, p0 in enumerate(range(0, P, 128)):
                    pe = min(P, p0 + 128)
                    t = wp.tile([pe - p0, Fr], BF, tag=f"{tag}_{ki}")
                    nc.sync.dma_start(t[:], dram[p0:pe, :])
                    parts.append(t)
                return parts

            WT = [load_bf(dWT[l], (IN if l == 0 else H), H, f"WT{l}") for l in range(4)]
            Wn = {l: load_bf(dWn[l], H, H, f"Wn{l}") for l in (1, 2, 3)}
            W0n = load_bf(dW0n, H, IN, "W0n")
            W0q = load_bf(dW0q, H, ND, "W0q")
            bs = []
            for l in range(4):
                ps_ = []
                for ki in range(KT):
                    t = wp.tile([128, 1], F32, tag=f"b{l}_{ki}")
                    nc.sync.dma_start(t[:], dbs[l][ki * 128:(ki + 1) * 128, :])
                    ps_.append(t)
                bs.append(ps_)
            w4t = []
            for ki in range(KT):
                t = wp.tile([128, 1], F32, tag=f"w4_{ki}")
                nc.sync.dma_start(t[:], dw4[ki * 128:(ki + 1) * 128, :])
                w4t.append(t)

            # W0q broadcast-materialized along T so tangent products stay packed
            W0qT = []
            for ki in range(KT):
                t = wp.tile([128, ND, T], BF, tag=f"W0qT_{ki}")
                nc.vector.tensor_scalar_mul(
                    t[:], W0q[ki][:].unsqueeze(2).broadcast_to((128, ND, T)), 1.0)
                W0qT.append(t)

            # inputs: X^T (bf16) for the forward pass, qdot tile for Coriolis
            XT = actp.tile([IN, N], BF)
            for g in range(NG):
                xt = xp.tile([128, IN], F32, tag="xt")
                nc.sync.dma_start(xt[:, 0:ND], dq[g * 128:(g + 1) * 128, :])
                nc.sync.dma_start(xt[:, ND:], dqd[g * 128:(g + 1) * 128, :])
                pt = psT.tile([IN, 128], F32, tag="ptx")
                nc.tensor.transpose(pt[:], xt[:], ident[:])
                nc.scalar.activation(XT[:, g * 128:(g + 1) * 128], pt[:], AF.Copy)
            Xqs = tlp.tile([128, NG, ND], F32)
            nc.sync.dma_start(
                Xqs[:], dqd[:].rearrange("(g p) r -> p g r", p=128))
            Xqd = tlp.tile([128, NG, ND], BF)
            nc.vector.tensor_scalar_mul(Xqd[:], Xqs[:], 1.0)

            # ---------------- phase 1: forward -------------------------------
            # A = W z + b (f32 psum); P=exp(-|A|); L=ln(1+P); S=exp(min(A,0)-L);
            # Z = relu(A)+L.  ACT: evac,P,L,S ; DVE: |A|, d, Z.
            Sbf = [[None] * KT for _ in range(4)]
            Zbf = [[None] * KT for _ in range(3)]

            def fwd_unit(l, ot, parts):
                ps = psB.tile([128, 1024], F32, tag="ps")
                nk = len(parts)
                for ki in range(nk):
                    for c0 in range(0, N, 512):
                        nc.tensor.matmul(ps[:, c0:c0 + 512],
                                         WT[l][ki][:, ot * 128:(ot + 1) * 128],
                                         parts[ki][:, c0:c0 + 512],
                                         start=(ki == 0), stop=(ki == nk - 1))
                A = ph1.tile([128, N], F32, tag=f"A{ot}")
                nc.scalar.activation(A[:], ps[:, 0:N], AF.Identity, bias=bs[l][ot][:])
                ab = ph1.tile([128, N], F32, tag=f"t1{ot}")
                nc.scalar.activation(ab[:], A[:], AF.Abs)
                P = ph1.tile([128, N], F32, tag=f"t2{ot}")
                nc.scalar.activation(P[:], ab[:], AF.Exp, scale=-1.0)
                L = ph1.tile([128, N], F32, tag=f"t1{ot}")
                nc.scalar.activation(L[:], P[:], AF.Ln, bias=1.0)
                d = ph1.tile([128, N], F32, tag=f"t2{ot}")
                nc.vector.scalar_tensor_tensor(d[:], A[:], 0.0, L[:], ALU.min, ALU.subtract)
                S = actp.tile([128, N], BF, tag=f"S{l}_{ot}")
                nc.scalar.activation(S[:], d[:], AF.Exp)
                Sbf[l][ot] = S
                if l < 3:
                    Z = actp.tile([128, N], BF, tag=f"Z{l}_{ot}")
                    nc.vector.scalar_tensor_tensor(Z[:], A[:], 0.0, L[:], ALU.max, ALU.add)
                    Zbf[l][ot] = Z

            for ot in range(KT):
                fwd_unit(0, ot, [XT[:]])
            for l in range(1, 4):
                for ot in range(KT):
                    fwd_unit(l, ot, [Zbf[l - 1][k][:] for k in range(KT)])

            # ---------------- phase 1: backward ------------------------------
            D4, c4 = [], []
            for ot in range(KT):
                Dt = actp.tile([128, N], BF, tag=f"D4_{ot}")
                nc.vector.tensor_scalar_mul(Dt[:], Sbf[3][ot][:], w4t[ot][:])
                D4.append(Dt)
                t1 = ph1.tile([128, N], BF, tag=f"bt{ot}")
                nc.vector.tensor_mul(t1[:], Dt[:], Sbf[3][ot][:])
                c = actp.tile([128, N], BF, tag=f"c4_{ot}")
                nc.vector.scalar_tensor_tensor(c[:], t1[:], -1.0, Dt[:], ALU.mult, ALU.add)
                c4.append(c)

            def bwd_unit(l, Dup, want_F):
                Ds, Fs = [], []
                for ot in range(KT):
                    ps = psB.tile([128, 1024], F32, tag="ps")
                    for ki in range(KT):
                        for c0 in range(0, N, 512):
                            nc.tensor.matmul(ps[:, c0:c0 + 512],
                                             Wn[l][ki][:, ot * 128:(ot + 1) * 128],
                                             Dup[ki][:, c0:c0 + 512],
                                             start=(ki == 0), stop=(ki == KT - 1))
                    U = ph1.tile([128, N], BF, tag=f"U{ot}")
                    nc.scalar.activation(U[:], ps[:, 0:N], AF.Copy)
                    D = actp.tile([128, N], BF, tag=f"D{l}_{ot}")
                    nc.vector.tensor_mul(D[:], Sbf[l - 1][ot][:], U[:])
                    Ds.append(D)
                    if want_F:
                        F = actp.tile([128, N], BF, tag=f"F{l}_{ot}")
                        nc.vector.scalar_tensor_tensor(F[:], D[:], -1.0, U[:], ALU.mult, ALU.add)
                        Fs.append(F)
                return Ds, Fs

            D3, F3 = bwd_unit(3, D4, True)
            D2, F2 = bwd_unit(2, D3, True)
            D1, _ = bwd_unit(1, D2, False)
            E1 = []
            for ot in range(KT):
                t1 = ph1.tile([128, N], BF, tag=f"bt{ot}")
                nc.vector.tensor_mul(t1[:], D1[ot][:], Sbf[0][ot][:])
                E = actp.tile([128, N], BF, tag=f"E1_{ot}")
                nc.vector.scalar_tensor_tensor(E[:], t1[:], -1.0, D1[ot][:], ALU.mult, ALU.add)
                E1.append(E)

            # g_q, pre-scaled by 100 for the Jacobi solve
            Gps = psB.tile([128, 1024], F32, tag="ps")
            for ki in range(KT):
                for c0 in range(0, N, 512):
                    nc.tensor.matmul(Gps[0:IN, c0:c0 + 512], W0n[ki][:],
                                     D1[ki][:, c0:c0 + 512],
                                     start=(ki == 0), stop=(ki == KT - 1))
            G = actp.tile([ND, N], F32)
            nc.scalar.activation(G[:], Gps[0:ND, 0:N], AF.Copy)
            Rt = tlp.tile([128, ND, NG], F32)
            for g in range(NG):
                ptg = psT.tile([128, IN], F32, tag="ptg")
                nc.tensor.transpose(ptg[:, 0:ND], G[:, g * 128:(g + 1) * 128],
                                    ident[0:ND, 0:ND])
                nc.scalar.activation(Rt[:, :, g], ptg[:, 0:ND], AF.Copy, scale=100.0)

            # ---------------- tangent blocks ---------------------------------
            # Ubf[p, g, d, r] = H[r, 12+d] for sample (g*128+p), r in 0..23
            Ubf = tlp.tile([128, NG, ND, 32], BF)
            Hcs = []
            for par in range(2):
                hct = hcp.tile([32, ND, 128], BF, tag=f"Hc{par}", name=f"Hc{par}")
                nc.gpsimd.memset(hct[:], 0.0)
                Hcs.append(hct)

            def flat(x):
                return x[:].rearrange("p d t -> p (d t)")

            for b in range(NT):
                g, off = b // 2, (b % 2) * T
                sl = slice(b * T, (b + 1) * T)

                def bca(Sten, ot):
                    return Sten[ot][:, sl].unsqueeze(1).broadcast_to((128, ND, T))

                def trans_mm(moving, lhs_parts, ot):
                    ps = psB.tile([128, 1024], F32, tag="ps")
                    for ki in range(KT):
                        mv = flat(moving[ki])
                        for c0 in range(0, FD, 512):
                            ce = min(FD, c0 + 512)
                            nc.tensor.matmul(ps[:, c0:ce],
                                             lhs_parts[ki][:, ot * 128:(ot + 1) * 128],
                                             mv[:, c0:ce],
                                             start=(ki == 0), stop=(ki == KT - 1))
                    ev = evp.tile([128, ND, T], BF, tag=f"ev{ot}")
                    nc.scalar.activation(flat(ev), ps[:, 0:FD], AF.Copy)
                    return ev

                Zd1, Zd2, Zd3, Dd4 = [], [], [], []
                for ot in range(KT):
                    z = tgp.tile([128, ND, T], BF, tag=f"Zd1_{ot}")
                    nc.vector.tensor_mul(z[:], bca(Sbf[0], ot), W0qT[ot][:])
                    Zd1.append(z)
                for ot in range(KT):
                    ev = trans_mm(Zd1, WT[1], ot)
                    z = tgp.tile([128, ND, T], BF, tag=f"Zd2_{ot}")
                    nc.vector.tensor_mul(z[:], bca(Sbf[1], ot), ev[:])
                    Zd2.append(z)
                for ot in range(KT):
                    ev = trans_mm(Zd2, WT[2], ot)
                    z = tgp.tile([128, ND, T], BF, tag=f"Zd3_{ot}")
                    nc.vector.tensor_mul(z[:], bca(Sbf[2], ot), ev[:])
                    Zd3.append(z)
                for ot in range(KT):
                    ev = trans_mm(Zd3, WT[3], ot)
                    dd = tgp.tile([128, ND, T], BF, tag=f"Dd4_{ot}")
                    nc.vector.tensor_mul(dd[:], bca(c4, ot), ev[:])
                    Dd4.append(dd)

                def tang_bwd(Dup, l, Sl, Fl, Zdl, dtag):
                    outs = []
                    for ot in range(KT):
                        ev = trans_mm(Dup, Wn[l], ot)
                        tb = tgp.tile([128, ND, T], BF, tag=f"tb{ot}")
                        nc.vector.tensor_mul(tb[:], bca(Sl, ot), ev[:])
                        ta = tgp.tile([128, ND, T], BF, tag=f"ta{ot}")
                        nc.vector.tensor_mul(ta[:], bca(Fl, ot), Zdl[ot][:])
                        dd = tgp.tile([128, ND, T], BF, tag=f"Dd{dtag}_{ot}")
                        nc.vector.scalar_tensor_tensor(dd[:], ta[:], 1.0, tb[:],
                                                       ALU.mult, ALU.add)
                        outs.append(dd)
                    return outs

                Dd3 = tang_bwd(Dd4, 3, Sbf[2], F3, Zd3, "3")
                Dd2 = tang_bwd(Dd3, 2, Sbf[1], F2, Zd2, "2")
                Dd1 = []
                for ot in range(KT):
                    ev = trans_mm(Dd2, Wn[1], ot)
                    tb = tgp.tile([128, ND, T], BF, tag=f"tb{ot}")
                    nc.vector.tensor_mul(tb[:], bca(Sbf[0], ot), ev[:])
                    ta = tgp.tile([128, ND, T], BF, tag=f"ta{ot}")
                    nc.vector.tensor_mul(ta[:], bca(E1, ot), W0qT[ot][:])
                    dd = tgp.tile([128, ND, T], BF, tag=f"Dd1_{ot}")
                    nc.vector.scalar_tensor_tensor(dd[:], ta[:], 1.0, tb[:],
                                                   ALU.mult, ALU.add)
                    Dd1.append(dd)

                psH = psB.tile([128, 1024], F32, tag="ps")
                for ki in range(KT):
                    mv = flat(Dd1[ki])
                    for c0 in range(0, FD, 512):
                        ce = min(FD, c0 + 512)
                        nc.tensor.matmul(psH[0:IN, c0:ce], W0n[ki][:], mv[:, c0:ce],
                                         start=(ki == 0), stop=(ki == KT - 1))
                Hc = Hcs[g % 2]
                hsl = Hc[0:IN, :, off:off + T]
                psr = psH[0:IN, 0:FD].rearrange("p (d t) -> p d t", d=ND)
                if b % 2 == 0:
                    nc.scalar.activation(hsl, psr, AF.Copy)
                else:
                    nc.vector.tensor_scalar_mul(hsl, psr, 1.0)

                if off == T:
                    nc.sync.dma_start(dhsc[g], Hc[:].rearrange("p d t -> p (d t)"))
                    nc.sync.dma_start_transpose(
                        out=Ubf[:, g, :, :], in_=dhsc[g])

            # ---------------- tail: Coriolis, rhs, Jacobi solve --------------
            # cor[g,d] = sum_r<12 Ubf[p,g,d,r] * qdot[p,g,r]
            ct = tlp.tile([128, NG, ND, ND], BF)
            nc.vector.tensor_tensor(
                ct[:], Ubf[:, :, :, 0:ND],
                Xqd[:].unsqueeze(2).broadcast_to((128, NG, ND, ND)), ALU.mult)
            corT = tlp.tile([128, NG, ND], F32)
            nc.vector.tensor_reduce(corT[:].unsqueeze(3), ct[:],
                                    axis=mybir.AxisListType.X, op=ALU.add)
            # r100[p,i,g] = 100*g_q - 100*cor
            r100 = tlp.tile([128, ND, NG], F32)
            nc.vector.scalar_tensor_tensor(
                r100[:], corT[:].rearrange("p g i -> p i g"), -100.0, Rt[:],
                ALU.mult, ALU.add)
            # M (bf16, packed j-contiguous): M[p,i,g,j] = Ubf[p,g,j,12+i]
            Mbf = tlp.tile([128, ND, NG, ND], BF)
            nc.scalar.activation(
                Mbf[:], Ubf[:, :, :, 12:24].rearrange("p g j i -> p i g j"),
                AF.Copy)
            # x0 = r100 ; x_{n+1} = r100 - 100*(H22 x_n)
            xs = []
            for it in range(2):
                xs.append(tlp.tile([128, NG, ND], BF, tag=f"x{it}", name=f"x{it}"))
            nc.vector.tensor_scalar_mul(
                xs[0][:], r100[:].rearrange("p i g -> p g i"), 1.0)
            xf = tlp.tile([128, NG, ND], F32)
            for it in range(JAC_ITERS):
                xin = xs[it % 2]
                tmp = tlp.tile([128, ND, NG, ND], BF, tag=f"jt{it % 2}")
                nc.vector.tensor_tensor(
                    tmp[:], Mbf[:],
                    xin[:].unsqueeze(1).broadcast_to((128, ND, NG, ND)), ALU.mult)
                y = tlp.tile([128, ND, NG], F32, tag=f"y{it % 2}")
                nc.vector.tensor_reduce(y[:].unsqueeze(3), tmp[:],
                                        axis=mybir.AxisListType.X, op=ALU.add)
                if it < JAC_ITERS - 1:
                    nc.vector.scalar_tensor_tensor(
                        xs[(it + 1) % 2][:], y[:].rearrange("p i g -> p g i"),
                        -100.0, r100[:].rearrange("p i g -> p g i"),
                        ALU.mult, ALU.add)
                else:
                    nc.vector.scalar_tensor_tensor(
                        xf[:], y[:].rearrange("p i g -> p g i"), -100.0,
                        r100[:].rearrange("p i g -> p g i"), ALU.mult, ALU.add)

            nc.sync.dma_start(
                dout[:].rearrange("(g p) i -> p g i", p=128), xf[:])

    nc.compile()
    return nc


def kernel(**inputs):
    q = np.ascontiguousarray(inputs["q"], dtype=np.float32)
    qdot = np.ascontiguousarray(inputs["qdot"], dtype=np.float32)
    if "nc" not in _cache:
        _cache["nc"] = build_kernel()
    nc = _cache["nc"]

    def b(x):
        return np.ascontiguousarray(x).astype(np.float32).astype(bf16)

    W0 = np.asarray(inputs["W0"], dtype=np.float32)
    base = {
        "WT0": b(W0.T),
        "WT1": b(np.asarray(inputs["W1"]).T),
        "WT2": b(np.asarray(inputs["W2"]).T),
        "WT3": b(np.asarray(inputs["W3"]).T),
        "Wn1": b(inputs["W1"]),
        "Wn2": b(inputs["W2"]),
        "Wn3": b(inputs["W3"]),
        "W0n": b(W0),
        "W0q": b(W0[:, ND:]),
        "b0": inputs["b0"].reshape(H, 1).astype(np.float32),
        "b1": inputs["b1"].reshape(H, 1).astype(np.float32),
        "b2": inputs["b2"].reshape(H, 1).astype(np.float32),
        "b3": inputs["b3"].reshape(H, 1).astype(np.float32),
        "w4": np.asarray(inputs["W4"]).reshape(H, 1).astype(np.float32),
        "ident": np.eye(128, dtype=np.float32),
    }
    in_maps = []
    for c in range(NC):
        m = dict(base)
        m["q"] = q[c * N:(c + 1) * N]
        m["qdot"] = qdot[c * N:(c + 1) * N]
        in_maps.append(m)
    res = run_bass_kernel_spmd(nc, in_maps, core_ids=list(range(NC)))
    _cache["res"] = res
    out = np.concatenate([res.results[c]["qdd"] for c in range(NC)], axis=0)
    return out.astype(np.float32)


# revision 19
# speedup vs baseline: 1.6497x; 1.2956x over previous
# Lagrangian-NN qddot kernel for TRN2 (8 NeuronCores, data-parallel over batch).
#
# Math: scalar L(q,qdot) = MLP(24->256x4->1, softplus). Per sample:
#   M = d2L/dqdot2 + 0.01 I ; C = d2L/dqdot dq ; qddot = M^-1 (dL/dq - C qdot).
# Batched fwd+bwd gives grad; 12 qdot-direction forward-over-reverse tangents give
# H[:,12:] whose symmetry supplies both M and the Coriolis contraction.
#
# v2: all matmuls bf16 (FastWeightLoad + 1024-wide moving operands); sigmoid/
# softplus chain in f32 on ACT {exp,ln} single act-table; tangent-phase PSUM
# evacuation on ACT (bf16 out) + DVE 2x products + 4x fused adds (stt);
# H layout transpose via XBAR DMA transpose (idle DMA engines); per-sample
# 12x12 solve by Jacobi iteration x <- 100(r - H22 x), contraction <= 0.035
# (M = 0.01(I+K), ||K|| small), 5 iters.
import sys
import numpy as np

for p in ("/opt/trn_rl_repo", "/root/.axon_site/_ro/trn_rl_repo"):
    if p not in sys.path:
        sys.path.insert(0, p)

import ml_dtypes
import concourse.bass as bass
import concourse.mybir as mybir
import concourse.tile as tile
from concourse import bacc
from concourse.bass_utils import run_bass_kernel_spmd

F32 = mybir.dt.float32
BF = mybir.dt.bfloat16
AF = mybir.ActivationFunctionType
ALU = mybir.AluOpType
bf16 = ml_dtypes.bfloat16

B, ND, H, NC = 8192, 12, 256, 8
N = B // NC          # 1024 samples per core
IN = 2 * ND          # 24
T = 64               # samples per tangent block
NT = N // T          # 16 blocks
NG = N // 128        # 8 groups of 128 samples
FD = ND * T          # 768 tangent free dim
KT = H // 128        # 2 partition chunks of the hidden dim
JAC_ITERS = 4

_cache = {}


def _patch_single_act_table():
    """Force every activation to resolve to natural_log_exp_and_others so a
    single ACT table load suffices (instead of ping-ponging exp/ln sets).
    Only the membership sets are modified; names/indices keep their original
    act_info.json order so emitted act_func_set_ids stay valid."""
    import concourse.bacc as bacc_mod
    from concourse.hw_specs import get_activation_tables as orig

    if getattr(bacc_mod, "_single_act_patch", False):
        return

    def patched(arch):
        t = orig(arch)
        return {k: (v if k == "natural_log_exp_and_others" else set())
                for k, v in t.items()}

    bacc_mod.get_activation_tables = patched
    bacc_mod._single_act_patch = True


def build_kernel():
    _patch_single_act_table()
    nc = bacc.Bacc("TRN2", target_bir_lowering=False)
    dq = nc.dram_tensor("q", (N, ND), F32, kind="ExternalInput")
    dqd = nc.dram_tensor("qdot", (N, ND), F32, kind="ExternalInput")
    dWT = [nc.dram_tensor(f"WT{l}", s, BF, kind="ExternalInput")
           for l, s in enumerate([(IN, H), (H, H), (H, H), (H, H)])]
    dWn = {l: nc.dram_tensor(f"Wn{l}", (H, H), BF, kind="ExternalInput") for l in (1, 2, 3)}
    dW0n = nc.dram_tensor("W0n", (H, IN), BF, kind="ExternalInput")
    dW0q = nc.dram_tensor("W0q", (H, ND), BF, kind="ExternalInput")
    dbs = [nc.dram_tensor(f"b{l}", (H, 1), F32, kind="ExternalInput") for l in range(4)]
    dw4 = nc.dram_tensor("w4", (H, 1), F32, kind="ExternalInput")
    dide = nc.dram_tensor("ident", (128, 128), F32, kind="ExternalInput")
    dhsc = nc.dram_tensor("hscratch", (NG, 32, ND * 128), BF, kind="Internal")
    dout = nc.dram_tensor("qdd", (N, ND), F32, kind="ExternalOutput")

    with tile.TileContext(nc) as tc:
        with tc.tile_pool(name="wp", bufs=1) as wp, \
             tc.tile_pool(name="ph1", bufs=1) as ph1, \
             tc.tile_pool(name="acts", bufs=1) as actp, \
             tc.tile_pool(name="evp", bufs=4) as evp, \
             tc.tile_pool(name="tang", bufs=2) as tgp, \
             tc.tile_pool(name="hcp", bufs=2) as hcp, \
             tc.tile_pool(name="tail", bufs=1) as tlp, \
             tc.tile_pool(name="xp", bufs=2) as xp, \
             tc.tile_pool(name="psB", bufs=4, space="PSUM") as psB:

            ident = wp.tile([128, 128], F32)
            nc.sync.dma_start(ident[:], dide[:])

            def load_bf(dram, P, Fr, tag):
                parts = []
                for ki, p0 in enumerate(range(0, P, 128)):
                    pe = min(P, p0 + 128)
                    t = wp.tile([pe - p0, Fr], BF, tag=f"{tag}_{ki}")
                    nc.sync.dma_start(t[:], dram[p0:pe, :])
                    parts.append(t)
                return parts

            WT = [load_bf(dWT[l], (IN if l == 0 else H), H, f"WT{l}") for l in range(4)]
            Wn = {l: load_bf(dWn[l], H, H, f"Wn{l}") for l in (1, 2, 3)}
            W0n = load_bf(dW0n, H, IN, "W0n")
            W0q = load_bf(dW0q, H, ND, "W0q")
            bs = []
            for l in range(4):
                ps_ = []
                for ki in range(KT):
                    t = wp.tile([128, 1], F32, tag=f"b{l}_{ki}")
                    nc.sync.dma_start(t[:], dbs[l][ki * 128:(ki + 1) * 128, :])
                    ps_.append(t)
                bs.append(ps_)
            w4t = []
            for ki in range(KT):
                t = wp.tile([128, 1], F32, tag=f"w4_{ki}")
                nc.sync.dma_start(t[:], dw4[ki * 128:(ki + 1) * 128, :])
                w4t.append(t)

            # W0q broadcast-materialized along T so tangent products stay packed
            W0qT = []
            for ki in range(KT):
                t = wp.tile([128, ND, T], BF, tag=f"W0qT_{ki}")
                nc.vector.tensor_scalar_mul(
                    t[:], W0q[ki][:].unsqueeze(2).broadcast_to((128, ND, T)), 1.0)
                W0qT.append(t)

            # inputs: X^T (bf16) for the forward pass, qdot tile for Coriolis
            XT = actp.tile([IN, N], BF)
            for g in range(NG):
                xt = xp.tile([128, IN], F32, tag="xt")
                nc.sync.dma_start(xt[:, 0:ND], dq[g * 128:(g + 1) * 128, :])
                nc.sync.dma_start(xt[:, ND:], dqd[g * 128:(g + 1) * 128, :])
                pt = psB.tile([128, 1024], F32, tag="ps", name="ptx")
                nc.tensor.transpose(pt[0:IN, 0:128], xt[:], ident[:])
                nc.vector.tensor_scalar_mul(XT[:, g * 128:(g + 1) * 128],
                                            pt[0:IN, 0:128], 1.0)
            Xqs = tlp.tile([128, NG, ND], F32)
            nc.sync.dma_start(
                Xqs[:], dqd[:].rearrange("(g p) r -> p g r", p=128))
            Xqd = tlp.tile([128, NG, ND], BF)
            nc.vector.tensor_scalar_mul(Xqd[:], Xqs[:], 1.0)

            # ---------------- phase 1: forward -------------------------------
            # A = W z + b (f32 psum); P=exp(-|A|); L=ln(1+P); S=exp(min(A,0)-L);
            # Z = relu(A)+L.  ACT: evac,P,L,S ; DVE: |A|, d, Z.
            Sbf = [[None] * KT for _ in range(4)]
            Zbf = [[None] * KT for _ in range(3)]

            def fwd_unit(l, ot, parts):
                ps = psB.tile([128, 1024], F32, tag="ps")
                nk = len(parts)
                for ki in range(nk):
                    for c0 in range(0, N, 512):
                        nc.tensor.matmul(ps[:, c0:c0 + 512],
                                         WT[l][ki][:, ot * 128:(ot + 1) * 128],
                                         parts[ki][:, c0:c0 + 512],
                                         start=(ki == 0), stop=(ki == nk - 1))
                A = ph1.tile([128, N], F32, tag=f"A{ot}")
                nc.scalar.activation(A[:], ps[:, 0:N], AF.Identity, bias=bs[l][ot][:])
                ab = ph1.tile([128, N], F32, tag=f"t1{ot}")
                nc.scalar.activation(ab[:], A[:], AF.Abs)
                P = ph1.tile([128, N], F32, tag=f"t2{ot}")
                nc.scalar.activation(P[:], ab[:], AF.Exp, scale=-1.0)
                L = ph1.tile([128, N], F32, tag=f"t1{ot}")
                nc.scalar.activation(L[:], P[:], AF.Ln, bias=1.0)
                d = ph1.tile([128, N], F32, tag=f"t2{ot}")
                nc.vector.scalar_tensor_tensor(d[:], A[:], 0.0, L[:], ALU.min, ALU.subtract)
                S = actp.tile([128, N], BF, tag=f"S{l}_{ot}")
                nc.scalar.activation(S[:], d[:], AF.Exp)
                Sbf[l][ot] = S
                if l < 3:
                    Z = actp.tile([128, N], BF, tag=f"Z{l}_{ot}")
                    nc.vector.scalar_tensor_tensor(Z[:], A[:], 0.0, L[:], ALU.max, ALU.add)
                    Zbf[l][ot] = Z

            for ot in range(KT):
                fwd_unit(0, ot, [XT[:]])
            for l in range(1, 4):
                for ot in range(KT):
                    fwd_unit(l, ot, [Zbf[l - 1][k][:] for k in range(KT)])

            # ---------------- phase 1: backward ------------------------------
            D4, c4 = [], []
            for ot in range(KT):
                Dt = actp.tile([128, N], BF, tag=f"D4_{ot}")
                nc.vector.tensor_scalar_mul(Dt[:], Sbf[3][ot][:], w4t[ot][:])
                D4.append(Dt)
                t1 = ph1.tile([128, N], BF, tag=f"bt{ot}")
                nc.vector.tensor_mul(t1[:], Dt[:], Sbf[3][ot][:])
                c = actp.tile([128, N], BF, tag=f"c4_{ot}")
                nc.vector.scalar_tensor_tensor(c[:], t1[:], -1.0, Dt[:], ALU.mult, ALU.add)
                c4.append(c)

            def bwd_unit(l, Dup, want_F):
                Ds, Fs = [], []
                for ot in range(KT):
                    ps = psB.tile([128, 1024], F32, tag="ps")
                    for ki in range(KT):
                        for c0 in range(0, N, 512):
                            nc.tensor.matmul(ps[:, c0:c0 + 512],
                                             Wn[l][ki][:, ot * 128:(ot + 1) * 128],
                                             Dup[ki][:, c0:c0 + 512],
                                             start=(ki == 0), stop=(ki == KT - 1))
                    U = ph1.tile([128, N], BF, tag=f"U{ot}")
                    nc.scalar.activation(U[:], ps[:, 0:N], AF.Copy)
                    D = actp.tile([128, N], BF, tag=f"D{l}_{ot}")
                    nc.vector.tensor_mul(D[:], Sbf[l - 1][ot][:], U[:])
                    Ds.append(D)
                    if want_F:
                        F = actp.tile([128, N], BF, tag=f"F{l}_{ot}")
                        nc.vector.scalar_tensor_tensor(F[:], D[:], -1.0, U[:], ALU.mult, ALU.add)
                        Fs.append(F)
                return Ds, Fs

            D3, F3 = bwd_unit(3, D4, True)
            D2, F2 = bwd_unit(2, D3, True)
            D1, _ = bwd_unit(1, D2, False)
            E1 = []
            for ot in range(KT):
                t1 = ph1.tile([128, N], BF, tag=f"bt{ot}")
                nc.vector.tensor_mul(t1[:], D1[ot][:], Sbf[0][ot][:])
                E = actp.tile([128, N], BF, tag=f"E1_{ot}")
                nc.vector.scalar_tensor_tensor(E[:], t1[:], -1.0, D1[ot][:], ALU.mult, ALU.add)
                E1.append(E)

            # g_q, pre-scaled by 100 for the Jacobi solve
            Gps = psB.tile([128, 1024], F32, tag="ps")
            for ki in range(KT):
                for c0 in range(0, N, 512):
                    nc.tensor.matmul(Gps[0:IN, c0:c0 + 512], W0n[ki][:],
                                     D1[ki][:, c0:c0 + 512],
                                     start=(ki == 0), stop=(ki == KT - 1))
            G = actp.tile([ND, N], F32)
            nc.scalar.activation(G[:], Gps[0:ND, 0:N], AF.Copy)
            Rt = tlp.tile([128, ND, NG], F32)
            for g in range(NG):
                ptg = psB.tile([128, 1024], F32, tag="ps", name="ptg")
                nc.tensor.transpose(ptg[:, 0:ND], G[:, g * 128:(g + 1) * 128],
                                    ident[0:ND, 0:ND])
                nc.vector.tensor_scalar_mul(Rt[:, :, g], ptg[:, 0:ND], 100.0)

            # ---------------- tangent blocks ---------------------------------
            # Ubf[p, g, d, r] = H[r, 12+d] for sample (g*128+p), r in 0..23
            Ubf = tlp.tile([128, NG, ND, 32], BF)
            Hcs = []
            for par in range(2):
                hct = hcp.tile([32, ND, 128], BF, tag=f"Hc{par}", name=f"Hc{par}")
                nc.gpsimd.memset(hct[:], 0.0)
                Hcs.append(hct)

            def flat(x):
                return x[:].rearrange("p d t -> p (d t)")

            for b in range(NT):
                g, off = b // 2, (b % 2) * T
                sl = slice(b * T, (b + 1) * T)

                def bca(Sten, ot):
                    return Sten[ot][:, sl].unsqueeze(1).broadcast_to((128, ND, T))

                def fmul(out_t, sten, ot, in_t):
                    nc.vector.tensor_tensor(flat(out_t), bca(sten, ot),
                                            flat(in_t), ALU.mult)

                def trans_mm(moving, lhs_parts, ot):
                    ps = psB.tile([128, 1024], F32, tag="ps")
                    for ki in range(KT):
                        mv = flat(moving[ki])
                        for c0 in range(0, FD, 512):
                            ce = min(FD, c0 + 512)
                            nc.tensor.matmul(ps[:, c0:ce],
                                             lhs_parts[ki][:, ot * 128:(ot + 1) * 128],
                                             mv[:, c0:ce],
                                             start=(ki == 0), stop=(ki == KT - 1))
                    ev = evp.tile([128, ND, T], BF, tag=f"ev{ot}")
                    nc.scalar.activation(flat(ev), ps[:, 0:FD], AF.Copy)
                    return ev

                Zd1, Zd2, Zd3, Dd4 = [], [], [], []
                for ot in range(KT):
                    z = tgp.tile([128, ND, T], BF, tag=f"Zd1_{ot}")
                    fmul(z, Sbf[0], ot, W0qT[ot])
                    Zd1.append(z)
                for ot in range(KT):
                    ev = trans_mm(Zd1, WT[1], ot)
                    z = tgp.tile([128, ND, T], BF, tag=f"Zd2_{ot}")
                    fmul(z, Sbf[1], ot, ev)
                    Zd2.append(z)
                for ot in range(KT):
                    ev = trans_mm(Zd2, WT[2], ot)
                    z = tgp.tile([128, ND, T], BF, tag=f"Zd3_{ot}")
                    fmul(z, Sbf[2], ot, ev)
                    Zd3.append(z)
                for ot in range(KT):
                    ev = trans_mm(Zd3, WT[3], ot)
                    dd = tgp.tile([128, ND, T], BF, tag=f"Dd4_{ot}")
                    fmul(dd, c4, ot, ev)
                    Dd4.append(dd)

                def tang_bwd(Dup, l, Sl, Fl, Zdl, dtag):
                    outs = []
                    for ot in range(KT):
                        ev = trans_mm(Dup, Wn[l], ot)
                        tb = tgp.tile([128, ND, T], BF, tag=f"tb{ot}")
                        fmul(tb, Sl, ot, ev)
                        ta = tgp.tile([128, ND, T], BF, tag=f"ta{ot}")
                        fmul(ta, Fl, ot, Zdl[ot])
                        dd = tgp.tile([128, ND, T], BF, tag=f"Dd{dtag}_{ot}")
                        nc.vector.tensor_add(flat(dd), flat(ta), flat(tb))
                        outs.append(dd)
                    return outs

                Dd3 = tang_bwd(Dd4, 3, Sbf[2], F3, Zd3, "3")
                Dd2 = tang_bwd(Dd3, 2, Sbf[1], F2, Zd2, "2")
                Dd1 = []
                for ot in range(KT):
                    ev = trans_mm(Dd2, Wn[1], ot)
                    tb = tgp.tile([128, ND, T], BF, tag=f"tb{ot}")
                    fmul(tb, Sbf[0], ot, ev)
                    ta = tgp.tile([128, ND, T], BF, tag=f"ta{ot}")
                    fmul(ta, E1, ot, W0qT[ot])
                    dd = tgp.tile([128, ND, T], BF, tag=f"Dd1_{ot}")
                    nc.vector.tensor_add(flat(dd), flat(ta), flat(tb))
                    Dd1.append(dd)

                psH = psB.tile([128, 1024], F32, tag="ps")
                for ki in range(KT):
                    mv = flat(Dd1[ki])
                    for c0 in range(0, FD, 512):
                        ce = min(FD, c0 + 512)
                        nc.tensor.matmul(psH[0:IN, c0:ce], W0n[ki][:], mv[:, c0:ce],
                                         start=(ki == 0), stop=(ki == KT - 1))
                Hc = Hcs[g % 2]
                hsl = Hc[0:IN, :, off:off + T]
                psr = psH[0:IN, 0:FD].rearrange("p (d t) -> p d t", d=ND)
                if b % 2 == 0:
                    nc.scalar.activation(hsl, psr, AF.Copy)
                else:
                    nc.vector.tensor_scalar_mul(hsl, psr, 1.0)

                if off == T:
                    nc.sync.dma_start(dhsc[g], Hc[:].rearrange("p d t -> p (d t)"))
                    nc.sync.dma_start_transpose(
                        out=Ubf[:, g, :, :], in_=dhsc[g])

            # ---------------- tail: Coriolis, rhs, Jacobi solve --------------
            # cor[g,d] = sum_r<12 Ubf[p,g,d,r] * qdot[p,g,r]
            ct = tlp.tile([128, NG, ND, ND], BF)
            nc.vector.tensor_tensor(
                ct[:], Ubf[:, :, :, 0:ND],
                Xqd[:].unsqueeze(2).broadcast_to((128, NG, ND, ND)), ALU.mult)
            corT = tlp.tile([128, NG, ND], F32)
            nc.vector.tensor_reduce(corT[:].unsqueeze(3), ct[:],
                                    axis=mybir.AxisListType.X, op=ALU.add)
            # r100[p,i,g] = 100*g_q - 100*cor
            r100 = tlp.tile([128, ND, NG], F32)
            nc.vector.scalar_tensor_tensor(
                r100[:], corT[:].rearrange("p g i -> p i g"), -100.0, Rt[:],
                ALU.mult, ALU.add)
            # M (bf16, packed j-contiguous): M[p,i,g,j] = Ubf[p,g,j,12+i]
            Mbf = tlp.tile([128, ND, NG, ND], BF)
            nc.scalar.activation(
                Mbf[:], Ubf[:, :, :, 12:24].rearrange("p g j i -> p i g j"),
                AF.Copy)
            # x0 = r100 ; x_{n+1} = r100 - 100*(H22 x_n)
            xs = []
            for it in range(2):
                xs.append(tlp.tile([128, NG, ND], BF, tag=f"x{it}", name=f"x{it}"))
            nc.vector.tensor_scalar_mul(
                xs[0][:], r100[:].rearrange("p i g -> p g i"), 1.0)
            xf = tlp.tile([128, NG, ND], F32)
            for it in range(JAC_ITERS):
                xin = xs[it % 2]
                tmp = tlp.tile([128, ND, NG, ND], BF, tag=f"jt{it % 2}")
                nc.vector.tensor_tensor(
                    tmp[:], Mbf[:],
                    xin[:].unsqueeze(1).broadcast_to((128, ND, NG, ND)), ALU.mult)
                y = tlp.tile([128, ND, NG], F32, tag=f"y{it % 2}")
                nc.vector.tensor_reduce(y[:].unsqueeze(3), tmp[:],
                                        axis=mybir.AxisListType.X, op=ALU.add)
                if it < JAC_ITERS - 1:
                    nc.vector.scalar_tensor_tensor(
                        xs[(it + 1) % 2][:], y[:].rearrange("p i g -> p g i"),
                        -100.0, r100[:].rearrange("p i g -> p g i"),
                        ALU.mult, ALU.add)
                else:
                    nc.vector.scalar_tensor_tensor(
                        xf[:], y[:].rearrange("p i g -> p g i"), -100.0,
                        r100[:].rearrange("p i g -> p g i"), ALU.mult, ALU.add)

            nc.sync.dma_start(
                dout[:].rearrange("(g p) i -> p g i", p=128), xf[:])

    nc.compile()
    return nc


def kernel(**inputs):
    q = np.ascontiguousarray(inputs["q"], dtype=np.float32)
    qdot = np.ascontiguousarray(inputs["qdot"], dtype=np.float32)
    if "nc" not in _cache:
        _cache["nc"] = build_kernel()
    nc = _cache["nc"]

    def b(x):
        return np.ascontiguousarray(x).astype(np.float32).astype(bf16)

    W0 = np.asarray(inputs["W0"], dtype=np.float32)
    base = {
        "WT0": b(W0.T),
        "WT1": b(np.asarray(inputs["W1"]).T),
        "WT2": b(np.asarray(inputs["W2"]).T),
        "WT3": b(np.asarray(inputs["W3"]).T),
        "Wn1": b(inputs["W1"]),
        "Wn2": b(inputs["W2"]),
        "Wn3": b(inputs["W3"]),
        "W0n": b(W0),
        "W0q": b(W0[:, ND:]),
        "b0": inputs["b0"].reshape(H, 1).astype(np.float32),
        "b1": inputs["b1"].reshape(H, 1).astype(np.float32),
        "b2": inputs["b2"].reshape(H, 1).astype(np.float32),
        "b3": inputs["b3"].reshape(H, 1).astype(np.float32),
        "w4": np.asarray(inputs["W4"]).reshape(H, 1).astype(np.float32),
        "ident": np.eye(128, dtype=np.float32),
    }
    in_maps = []
    for c in range(NC):
        m = dict(base)
        m["q"] = q[c * N:(c + 1) * N]
        m["qdot"] = qdot[c * N:(c + 1) * N]
        in_maps.append(m)
    res = run_bass_kernel_spmd(nc, in_maps, core_ids=list(range(NC)))
    _cache["res"] = res
    out = np.concatenate([res.results[c]["qdd"] for c in range(NC)], axis=0)
    return out.astype(np.float32)


# revision 20
# speedup vs baseline: 1.7381x; 1.0536x over previous
# Lagrangian-NN qddot kernel for TRN2 (8 NeuronCores, data-parallel over batch).
#
# Math: scalar L(q,qdot) = MLP(24->256x4->1, softplus). Per sample:
#   M = d2L/dqdot2 + 0.01 I ; C = d2L/dqdot dq ; qddot = M^-1 (dL/dq - C qdot).
# Batched fwd+bwd gives grad; 12 qdot-direction forward-over-reverse tangents give
# H[:,12:] whose symmetry supplies both M and the Coriolis contraction.
#
# v2: all matmuls bf16 (FastWeightLoad + 1024-wide moving operands); sigmoid/
# softplus chain in f32 on ACT {exp,ln} single act-table; tangent-phase PSUM
# evacuation on ACT (bf16 out) + DVE 2x products + 4x fused adds (stt);
# H layout transpose via XBAR DMA transpose (idle DMA engines); per-sample
# 12x12 solve by Jacobi iteration x <- 100(r - H22 x), contraction <= 0.035
# (M = 0.01(I+K), ||K|| small), 5 iters.
import sys
import numpy as np

for p in ("/opt/trn_rl_repo", "/root/.axon_site/_ro/trn_rl_repo"):
    if p not in sys.path:
        sys.path.insert(0, p)

import ml_dtypes
import concourse.bass as bass
import concourse.mybir as mybir
import concourse.tile as tile
from concourse import bacc
from concourse.bass_utils import run_bass_kernel_spmd

F32 = mybir.dt.float32
BF = mybir.dt.bfloat16
AF = mybir.ActivationFunctionType
ALU = mybir.AluOpType
bf16 = ml_dtypes.bfloat16

B, ND, H, NC = 8192, 12, 256, 8
N = B // NC          # 1024 samples per core
IN = 2 * ND          # 24
T = 64               # samples per tangent block
NT = N // T          # 16 blocks
NG = N // 128        # 8 groups of 128 samples
FD = ND * T          # 768 tangent free dim
KT = H // 128        # 2 partition chunks of the hidden dim
JAC_ITERS = 4

_cache = {}


def _patch_single_act_table():
    """Force every activation to resolve to natural_log_exp_and_others so a
    single ACT table load suffices (instead of ping-ponging exp/ln sets).
    Only the membership sets are modified; names/indices keep their original
    act_info.json order so emitted act_func_set_ids stay valid."""
    import concourse.bacc as bacc_mod
    from concourse.hw_specs import get_activation_tables as orig

    if getattr(bacc_mod, "_single_act_patch", False):
        return

    def patched(arch):
        t = orig(arch)
        return {k: (v if k == "natural_log_exp_and_others" else set())
                for k, v in t.items()}

    bacc_mod.get_activation_tables = patched
    bacc_mod._single_act_patch = True


def build_kernel():
    _patch_single_act_table()
    nc = bacc.Bacc("TRN2", target_bir_lowering=False)
    dq = nc.dram_tensor("q", (N, ND), F32, kind="ExternalInput")
    dqd = nc.dram_tensor("qdot", (N, ND), F32, kind="ExternalInput")
    dWT = [nc.dram_tensor(f"WT{l}", s, BF, kind="ExternalInput")
           for l, s in enumerate([(IN, H), (H, H), (H, H), (H, H)])]
    dWn = {l: nc.dram_tensor(f"Wn{l}", (H, H), BF, kind="ExternalInput") for l in (1, 2, 3)}
    dW0n = nc.dram_tensor("W0n", (H, IN), BF, kind="ExternalInput")
    dW0q = nc.dram_tensor("W0q", (H, ND), BF, kind="ExternalInput")
    dbs = [nc.dram_tensor(f"b{l}", (H, 1), F32, kind="ExternalInput") for l in range(4)]
    dw4 = nc.dram_tensor("w4", (H, 1), F32, kind="ExternalInput")
    dide = nc.dram_tensor("ident", (128, 128), F32, kind="ExternalInput")
    dhsc = nc.dram_tensor("hscratch", (NG, 32, ND * 128), BF, kind="Internal")
    dout = nc.dram_tensor("qdd", (N, ND), F32, kind="ExternalOutput")

    with tile.TileContext(nc) as tc:
        with tc.tile_pool(name="wp", bufs=1) as wp, \
             tc.tile_pool(name="ph1", bufs=1) as ph1, \
             tc.tile_pool(name="acts", bufs=1) as actp, \
             tc.tile_pool(name="evp", bufs=2) as evp, \
             tc.tile_pool(name="tang", bufs=1) as tgp, \
             tc.tile_pool(name="hcp", bufs=2) as hcp, \
             tc.tile_pool(name="tail", bufs=1) as tlp, \
             tc.tile_pool(name="xp", bufs=2) as xp, \
             tc.tile_pool(name="psB", bufs=4, space="PSUM") as psB:

            ident = wp.tile([128, 128], F32)
            nc.sync.dma_start(ident[:], dide[:])

            def load_bf(dram, P, Fr, tag):
                parts = []
                for ki, p0 in enumerate(range(0, P, 128)):
                    pe = min(P, p0 + 128)
                    t = wp.tile([pe - p0, Fr], BF, tag=f"{tag}_{ki}")
                    nc.sync.dma_start(t[:], dram[p0:pe, :])
                    parts.append(t)
                return parts

            WT = [load_bf(dWT[l], (IN if l == 0 else H), H, f"WT{l}") for l in range(4)]
            Wn = {l: load_bf(dWn[l], H, H, f"Wn{l}") for l in (1, 2, 3)}
            W0n = load_bf(dW0n, H, IN, "W0n")
            W0q = load_bf(dW0q, H, ND, "W0q")
            bs = []
            for l in range(4):
                ps_ = []
                for ki in range(KT):
                    t = wp.tile([128, 1], F32, tag=f"b{l}_{ki}")
                    nc.sync.dma_start(t[:], dbs[l][ki * 128:(ki + 1) * 128, :])
                    ps_.append(t)
                bs.append(ps_)
            w4t = []
            for ki in range(KT):
                t = wp.tile([128, 1], F32, tag=f"w4_{ki}")
                nc.sync.dma_start(t[:], dw4[ki * 128:(ki + 1) * 128, :])
                w4t.append(t)

            # W0q broadcast-materialized along T so tangent products stay packed
            W0qT = []
            for ki in range(KT):
                t = wp.tile([128, ND, T], BF, tag=f"W0qT_{ki}")
                nc.vector.tensor_scalar_mul(
                    t[:], W0q[ki][:].unsqueeze(2).broadcast_to((128, ND, T)), 1.0)
                W0qT.append(t)

            # inputs: X^T (bf16) for the forward pass, qdot tile for Coriolis
            XT = actp.tile([IN, N], BF)
            for g in range(NG):
                xt = xp.tile([128, IN], F32, tag="xt")
                nc.sync.dma_start(xt[:, 0:ND], dq[g * 128:(g + 1) * 128, :])
                nc.sync.dma_start(xt[:, ND:], dqd[g * 128:(g + 1) * 128, :])
                pt = psB.tile([128, 1024], F32, tag="ps", name="ptx")
                nc.tensor.transpose(pt[0:IN, 0:128], xt[:], ident[:])
                nc.vector.tensor_scalar_mul(XT[:, g * 128:(g + 1) * 128],
                                            pt[0:IN, 0:128], 1.0)
            Xqs = tlp.tile([128, NG, ND], F32)
            nc.sync.dma_start(
                Xqs[:], dqd[:].rearrange("(g p) r -> p g r", p=128))
            Xqd = tlp.tile([128, NG, ND], BF)
            nc.vector.tensor_scalar_mul(Xqd[:], Xqs[:], 1.0)

            # ---------------- phase 1: forward -------------------------------
            # A = W z + b (f32 psum); P=exp(-|A|); L=ln(1+P); S=exp(min(A,0)-L);
            # Z = relu(A)+L.  ACT: evac,P,L,S ; DVE: |A|, d, Z.
            Sbf = [[None] * KT for _ in range(4)]
            Zbf = [[None] * KT for _ in range(3)]

            def fwd_unit(l, ot, parts):
                ps = psB.tile([128, 1024], F32, tag="ps")
                nk = len(parts)
                for ki in range(nk):
                    for c0 in range(0, N, 512):
                        nc.tensor.matmul(ps[:, c0:c0 + 512],
                                         WT[l][ki][:, ot * 128:(ot + 1) * 128],
                                         parts[ki][:, c0:c0 + 512],
                                         start=(ki == 0), stop=(ki == nk - 1))
                A = ph1.tile([128, N], F32, tag=f"A{ot}")
                nc.scalar.activation(A[:], ps[:, 0:N], AF.Identity, bias=bs[l][ot][:])
                ab = ph1.tile([128, N], F32, tag=f"t1{ot}")
                nc.scalar.activation(ab[:], A[:], AF.Abs)
                P = ph1.tile([128, N], F32, tag=f"t2{ot}")
                nc.scalar.activation(P[:], ab[:], AF.Exp, scale=-1.0)
                L = ph1.tile([128, N], F32, tag=f"t1{ot}")
                nc.scalar.activation(L[:], P[:], AF.Ln, bias=1.0)
                d = ph1.tile([128, N], F32, tag=f"t2{ot}")
                nc.vector.scalar_tensor_tensor(d[:], A[:], 0.0, L[:], ALU.min, ALU.subtract)
                S = actp.tile([128, N], BF, tag=f"S{l}_{ot}")
                nc.scalar.activation(S[:], d[:], AF.Exp)
                Sbf[l][ot] = S
                if l < 3:
                    Z = actp.tile([128, N], BF, tag=f"Z{l}_{ot}")
                    nc.vector.scalar_tensor_tensor(Z[:], A[:], 0.0, L[:], ALU.max, ALU.add)
                    Zbf[l][ot] = Z

            for ot in range(KT):
                fwd_unit(0, ot, [XT[:]])
            for l in range(1, 4):
                for ot in range(KT):
                    fwd_unit(l, ot, [Zbf[l - 1][k][:] for k in range(KT)])

            # ---------------- phase 1: backward ------------------------------
            D4, c4 = [], []
            for ot in range(KT):
                Dt = actp.tile([128, N], BF, tag=f"D4_{ot}")
                nc.vector.tensor_scalar_mul(Dt[:], Sbf[3][ot][:], w4t[ot][:])
                D4.append(Dt)
                t1 = ph1.tile([128, N], BF, tag=f"bt{ot}")
                nc.vector.tensor_mul(t1[:], Dt[:], Sbf[3][ot][:])
                c = actp.tile([128, N], BF, tag=f"c4_{ot}")
                nc.vector.scalar_tensor_tensor(c[:], t1[:], -1.0, Dt[:], ALU.mult, ALU.add)
                c4.append(c)

            def bwd_unit(l, Dup, want_F):
                Ds, Fs = [], []
                for ot in range(KT):
                    ps = psB.tile([128, 1024], F32, tag="ps")
                    for ki in range(KT):
                        for c0 in range(0, N, 512):
                            nc.tensor.matmul(ps[:, c0:c0 + 512],
                                             Wn[l][ki][:, ot * 128:(ot + 1) * 128],
                                             Dup[ki][:, c0:c0 + 512],
                                             start=(ki == 0), stop=(ki == KT - 1))
                    U = ph1.tile([128, N], BF, tag=f"U{ot}")
                    nc.scalar.activation(U[:], ps[:, 0:N], AF.Copy)
                    D = actp.tile([128, N], BF, tag=f"D{l}_{ot}")
                    nc.vector.tensor_mul(D[:], Sbf[l - 1][ot][:], U[:])
                    Ds.append(D)
                    if want_F:
                        F = actp.tile([128, N], BF, tag=f"F{l}_{ot}")
                        nc.vector.scalar_tensor_tensor(F[:], D[:], -1.0, U[:], ALU.mult, ALU.add)
                        Fs.append(F)
                return Ds, Fs

            D3, F3 = bwd_unit(3, D4, True)
            D2, F2 = bwd_unit(2, D3, True)
            D1, _ = bwd_unit(1, D2, False)
            E1 = []
            for ot in range(KT):
                t1 = ph1.tile([128, N], BF, tag=f"bt{ot}")
                nc.vector.tensor_mul(t1[:], D1[ot][:], Sbf[0][ot][:])
                E = actp.tile([128, N], BF, tag=f"E1_{ot}")
                nc.vector.scalar_tensor_tensor(E[:], t1[:], -1.0, D1[ot][:], ALU.mult, ALU.add)
                E1.append(E)

            # g_q, pre-scaled by 100 for the Jacobi solve
            Gps = psB.tile([128, 1024], F32, tag="ps")
            for ki in range(KT):
                for c0 in range(0, N, 512):
                    nc.tensor.matmul(Gps[0:IN, c0:c0 + 512], W0n[ki][:],
                                     D1[ki][:, c0:c0 + 512],
                                     start=(ki == 0), stop=(ki == KT - 1))
            G = actp.tile([ND, N], F32)
            nc.scalar.activation(G[:], Gps[0:ND, 0:N], AF.Copy)
            Rt = tlp.tile([128, ND, NG], F32)
            for g in range(NG):
                ptg = psB.tile([128, 1024], F32, tag="ps", name="ptg")
                nc.tensor.transpose(ptg[:, 0:ND], G[:, g * 128:(g + 1) * 128],
                                    ident[0:ND, 0:ND])
                nc.vector.tensor_scalar_mul(Rt[:, :, g], ptg[:, 0:ND], 100.0)

            # ---------------- tangent blocks ---------------------------------
            # Ubf[p, g, d, r] = H[r, 12+d] for sample (g*128+p), r in 0..23
            Ubf = tlp.tile([128, NG, ND, 32], BF)
            Hcs = []
            for par in range(2):
                hct = hcp.tile([32, ND, 128], BF, tag=f"Hc{par}", name=f"Hc{par}")
                nc.gpsimd.memset(hct[:], 0.0)
                Hcs.append(hct)

            def flat(x):
                return x[:].rearrange("p d t -> p (d t)")

            for pb in range(0, NT, 2):
                bs2 = (pb, pb + 1)
                g = pb // 2
                sls = {b: slice(b * T, (b + 1) * T) for b in bs2}

                def bca(Sten, ot, b):
                    return Sten[ot][:, sls[b]].unsqueeze(1).broadcast_to((128, ND, T))

                def fmul(out_t, sten, ot, in_t, b):
                    nc.vector.tensor_tensor(flat(out_t), bca(sten, ot, b),
                                            flat(in_t), ALU.mult)

                def mk(tag, b):
                    return tgp.tile([128, ND, T], BF, tag=f"{tag}_{b % 2}", name=tag)

                def mm_step(moving, lhs_parts, ot):
                    ps = psB.tile([128, 1024], F32, tag="ps", name="tps")
                    for ki in range(KT):
                        mv = flat(moving[ki])
                        for c0 in range(0, FD, 512):
                            ce = min(FD, c0 + 512)
                            nc.tensor.matmul(ps[:, c0:ce],
                                             lhs_parts[ki][:, ot * 128:(ot + 1) * 128],
                                             mv[:, c0:ce],
                                             start=(ki == 0), stop=(ki == KT - 1))
                    return ps

                def evac_step(ps, ot, b):
                    ev = evp.tile([128, ND, T], BF, tag=f"ev{ot}_{b % 2}", name="ev")
                    nc.scalar.activation(flat(ev), ps[:, 0:FD], AF.Copy)
                    return ev

                st = {b: {} for b in bs2}

                for b in bs2:
                    for ot in range(KT):
                        z = mk(f"Zd1_{ot}", b)
                        fmul(z, Sbf[0], ot, W0qT[ot], b)
                        st[b][f"Zd1_{ot}"] = z

                for (src, dst, Wl, Sl) in (("Zd1", "Zd2", WT[1], Sbf[1]),
                                           ("Zd2", "Zd3", WT[2], Sbf[2]),
                                           ("Zd3", "Dd4", WT[3], c4)):
                    pss = {}
                    for b in bs2:
                        for ot in range(KT):
                            pss[(b, ot)] = mm_step(
                                [st[b][f"{src}_{k}"] for k in range(KT)], Wl, ot)
                    evs = {}
                    for b in bs2:
                        for ot in range(KT):
                            evs[(b, ot)] = evac_step(pss[(b, ot)], ot, b)
                    for b in bs2:
                        for ot in range(KT):
                            z = mk(f"{dst}_{ot}", b)
                            fmul(z, Sl, ot, evs[(b, ot)], b)
                            st[b][f"{dst}_{ot}"] = z

                for (src, dst, l, Sl, Fl, zsrc) in (
                        ("Dd4", "Dd3", 3, Sbf[2], F3, "Zd3"),
                        ("Dd3", "Dd2", 2, Sbf[1], F2, "Zd2"),
                        ("Dd2", "Dd1", 1, Sbf[0], E1, None)):
                    pss = {}
                    for b in bs2:
                        for ot in range(KT):
                            pss[(b, ot)] = mm_step(
                                [st[b][f"{src}_{k}"] for k in range(KT)], Wn[l], ot)
                    evs = {}
                    for b in bs2:
                        for ot in range(KT):
                            evs[(b, ot)] = evac_step(pss[(b, ot)], ot, b)
                    for b in bs2:
                        for ot in range(KT):
                            tb = mk(f"tb{ot}", b)
                            fmul(tb, Sl, ot, evs[(b, ot)], b)
                            ta = mk(f"ta{ot}", b)
                            if zsrc is None:
                                fmul(ta, Fl, ot, W0qT[ot], b)
                            else:
                                fmul(ta, Fl, ot, st[b][f"{zsrc}_{ot}"], b)
                            dd = mk(f"{dst}_{ot}", b)
                            nc.gpsimd.tensor_add(flat(dd), flat(ta), flat(tb))
                            st[b][f"{dst}_{ot}"] = dd

                Hc = Hcs[g % 2]
                psHs = {}
                for b in bs2:
                    psH = psB.tile([128, 1024], F32, tag="ps", name="psH")
                    for ki in range(KT):
                        mv = flat(st[b][f"Dd1_{ki}"])
                        for c0 in range(0, FD, 512):
                            ce = min(FD, c0 + 512)
                            nc.tensor.matmul(psH[0:IN, c0:ce], W0n[ki][:],
                                             mv[:, c0:ce],
                                             start=(ki == 0), stop=(ki == KT - 1))
                    psHs[b] = psH
                for b in bs2:
                    off = (b % 2) * T
                    hsl = Hc[0:IN, :, off:off + T]
                    psr = psHs[b][0:IN, 0:FD].rearrange("p (d t) -> p d t", d=ND)
                    if b % 2 == 0:
                        nc.scalar.activation(hsl, psr, AF.Copy)
                    else:
                        nc.vector.tensor_scalar_mul(hsl, psr, 1.0)
                nc.sync.dma_start(dhsc[g], Hc[:].rearrange("p d t -> p (d t)"))
                nc.sync.dma_start_transpose(out=Ubf[:, g, :, :], in_=dhsc[g])

            # ---------------- tail: Coriolis, rhs, Jacobi solve --------------
            # cor[g,d] = sum_r<12 Ubf[p,g,d,r] * qdot[p,g,r]
            ct = tlp.tile([128, NG, ND, ND], BF)
            nc.vector.tensor_tensor(
                ct[:], Ubf[:, :, :, 0:ND],
                Xqd[:].unsqueeze(2).broadcast_to((128, NG, ND, ND)), ALU.mult)
            corT = tlp.tile([128, NG, ND], F32)
            nc.vector.tensor_reduce(corT[:].unsqueeze(3), ct[:],
                                    axis=mybir.AxisListType.X, op=ALU.add)
            # r100[p,i,g] = 100*g_q - 100*cor
            r100 = tlp.tile([128, ND, NG], F32)
            nc.vector.scalar_tensor_tensor(
                r100[:], corT[:].rearrange("p g i -> p i g"), -100.0, Rt[:],
                ALU.mult, ALU.add)
            # M (bf16, packed j-contiguous): M[p,i,g,j] = Ubf[p,g,j,12+i]
            Mbf = tlp.tile([128, ND, NG, ND], BF)
            nc.scalar.activation(
                Mbf[:], Ubf[:, :, :, 12:24].rearrange("p g j i -> p i g j"),
                AF.Copy)
            # x0 = r100 ; x_{n+1} = r100 - 100*(H22 x_n)
            xs = []
            for it in range(2):
                xs.append(tlp.tile([128, NG, ND], BF, tag=f"x{it}", name=f"x{it}"))
            nc.vector.tensor_scalar_mul(
                xs[0][:], r100[:].rearrange("p i g -> p g i"), 1.0)
            xf = tlp.tile([128, NG, ND], F32)
            for it in range(JAC_ITERS):
                xin = xs[it % 2]
                tmp = tlp.tile([128, ND, NG, ND], BF, tag=f"jt{it % 2}")
                nc.vector.tensor_tensor(
                    tmp[:], Mbf[:],
                    xin[:].unsqueeze(1).broadcast_to((128, ND, NG, ND)), ALU.mult)
                y = tlp.tile([128, ND, NG], F32, tag=f"y{it % 2}")
                nc.vector.tensor_reduce(y[:].unsqueeze(3), tmp[:],
                                        axis=mybir.AxisListType.X, op=ALU.add)
                if it < JAC_ITERS - 1:
                    nc.vector.scalar_tensor_tensor(
                        xs[(it + 1) % 2][:], y[:].rearrange("p i g -> p g i"),
                        -100.0, r100[:].rearrange("p i g -> p g i"),
                        ALU.mult, ALU.add)
                else:
                    nc.vector.scalar_tensor_tensor(
                        xf[:], y[:].rearrange("p i g -> p g i"), -100.0,
                        r100[:].rearrange("p i g -> p g i"), ALU.mult, ALU.add)

            nc.sync.dma_start(
                dout[:].rearrange("(g p) i -> p g i", p=128), xf[:])

    nc.compile()
    return nc


def kernel(**inputs):
    q = np.ascontiguousarray(inputs["q"], dtype=np.float32)
    qdot = np.ascontiguousarray(inputs["qdot"], dtype=np.float32)
    if "nc" not in _cache:
        _cache["nc"] = build_kernel()
    nc = _cache["nc"]

    def b(x):
        return np.ascontiguousarray(x).astype(np.float32).astype(bf16)

    W0 = np.asarray(inputs["W0"], dtype=np.float32)
    base = {
        "WT0": b(W0.T),
        "WT1": b(np.asarray(inputs["W1"]).T),
        "WT2": b(np.asarray(inputs["W2"]).T),
        "WT3": b(np.asarray(inputs["W3"]).T),
        "Wn1": b(inputs["W1"]),
        "Wn2": b(inputs["W2"]),
        "Wn3": b(inputs["W3"]),
        "W0n": b(W0),
        "W0q": b(W0[:, ND:]),
        "b0": inputs["b0"].reshape(H, 1).astype(np.float32),
        "b1": inputs["b1"].reshape(H, 1).astype(np.float32),
        "b2": inputs["b2"].reshape(H, 1).astype(np.float32),
        "b3": inputs["b3"].reshape(H, 1).astype(np.float32),
        "w4": np.asarray(inputs["W4"]).reshape(H, 1).astype(np.float32),
        "ident": np.eye(128, dtype=np.float32),
    }
    in_maps = []
    for c in range(NC):
        m = dict(base)
        m["q"] = q[c * N:(c + 1) * N]
        m["qdot"] = qdot[c * N:(c + 1) * N]
        in_maps.append(m)
    res = run_bass_kernel_spmd(nc, in_maps, core_ids=list(range(NC)))
    _cache["res"] = res
    out = np.concatenate([res.results[c]["qdd"] for c in range(NC)], axis=0)
    return out.astype(np.float32)


# revision 24
# speedup vs baseline: 1.9162x; 1.1025x over previous
# Lagrangian-NN qddot kernel for TRN2 (8 NeuronCores, data-parallel over batch).
#
# Math: scalar L(q,qdot) = MLP(24->256x4->1, softplus). Per sample:
#   M = d2L/dqdot2 + 0.01 I ; C = d2L/dqdot dq ; qddot = M^-1 (dL/dq - C qdot).
# Batched fwd+bwd gives grad; 12 qdot-direction forward-over-reverse tangents give
# H[:,12:] whose symmetry supplies both M and the Coriolis contraction.
#
# v5: all matmuls bf16 (FastWeightLoad); phase-1 split into two half-batches
# overlapped with the tangent blocks; tangent blocks software-pipelined in
# pairs (PSUM depth 4); PSUM evacuation on ACT, products on DVE 2x (flat APs);
# backward-tangent adds folded into PSUM accumulation (ta/tb both matmul
# moving operands); H layout transposed via XBAR DMA through a DRAM scratch;
# per-sample 12x12 solve by Jacobi iteration x <- 100(r - H22 x) (contraction
# <= 0.035), 4 iters.
import sys
import numpy as np

for p in ("/opt/trn_rl_repo", "/root/.axon_site/_ro/trn_rl_repo"):
    if p not in sys.path:
        sys.path.insert(0, p)

import ml_dtypes
import concourse.bass as bass
import concourse.mybir as mybir
import concourse.tile as tile
from concourse import bacc
from concourse.bass_utils import run_bass_kernel_spmd

F32 = mybir.dt.float32
BF = mybir.dt.bfloat16
AF = mybir.ActivationFunctionType
ALU = mybir.AluOpType
bf16 = ml_dtypes.bfloat16

B, ND, H, NC = 8192, 12, 256, 8
N = B // NC          # 1024 samples per core
NH = N // 2          # 512 per phase-1 half
IN = 2 * ND          # 24
T = 64               # samples per tangent block
NT = N // T          # 16 blocks
NG = N // 128        # 8 groups of 128 samples
FD = ND * T          # 768 tangent free dim
KT = H // 128        # 2 partition chunks of the hidden dim
JAC_ITERS = 4

_cache = {}


def _patch_single_act_table():
    """Force every activation to resolve to natural_log_exp_and_others so a
    single ACT table load suffices (instead of ping-ponging exp/ln sets)."""
    import concourse.bacc as bacc_mod
    from concourse.hw_specs import get_activation_tables as orig

    if getattr(bacc_mod, "_single_act_patch", False):
        return

    def patched(arch):
        t = orig(arch)
        return {k: (v if k == "natural_log_exp_and_others" else set())
                for k, v in t.items()}

    bacc_mod.get_activation_tables = patched
    bacc_mod._single_act_patch = True


def build_kernel():
    _patch_single_act_table()
    nc = bacc.Bacc("TRN2", target_bir_lowering=False)
    dq = nc.dram_tensor("q", (N, ND), F32, kind="ExternalInput")
    dqd = nc.dram_tensor("qdot", (N, ND), F32, kind="ExternalInput")
    dWT = [nc.dram_tensor(f"WT{l}", s, BF, kind="ExternalInput")
           for l, s in enumerate([(IN, H), (H, H), (H, H), (H, H)])]
    dWn = {l: nc.dram_tensor(f"Wn{l}", (H, H), BF, kind="ExternalInput")
           for l in (1, 2, 3)}
    dW0n = nc.dram_tensor("W0n", (H, IN), BF, kind="ExternalInput")
    dW0q = nc.dram_tensor("W0q", (H, ND), BF, kind="ExternalInput")
    dbs = [nc.dram_tensor(f"b{l}", (H, 1), F32, kind="ExternalInput")
           for l in range(4)]
    dw4 = nc.dram_tensor("w4", (H, 1), F32, kind="ExternalInput")
    dide = nc.dram_tensor("ident", (128, 128), F32, kind="ExternalInput")
    dhsc = nc.dram_tensor("hscratch", (NG, 32, ND * 128), BF, kind="Internal")
    dout = nc.dram_tensor("qdd", (N, ND), F32, kind="ExternalOutput")

    with tile.TileContext(nc) as tc:
        with tc.tile_pool(name="wp", bufs=1) as wp, \
             tc.tile_pool(name="ph1", bufs=1) as ph1, \
             tc.tile_pool(name="acts", bufs=1) as actp, \
             tc.tile_pool(name="evp", bufs=2) as evp, \
             tc.tile_pool(name="tang", bufs=1) as tgp, \
             tc.tile_pool(name="zdp", bufs=2) as zdp, \
             tc.tile_pool(name="hcp", bufs=1) as hcp, \
             tc.tile_pool(name="tail", bufs=1) as tlp, \
             tc.tile_pool(name="xp", bufs=2) as xp, \
             tc.tile_pool(name="psB", bufs=4, space="PSUM") as psB:

            ident = wp.tile([128, 128], F32)
            nc.sync.dma_start(ident[:], dide[:])

            def load_bf(dram, P, Fr, tag):
                parts = []
                for ki, p0 in enumerate(range(0, P, 128)):
                    pe = min(P, p0 + 128)
                    t = wp.tile([pe - p0, Fr], BF, tag=f"{tag}_{ki}")
                    nc.sync.dma_start(t[:], dram[p0:pe, :])
                    parts.append(t)
                return parts

            WT = [load_bf(dWT[l], (IN if l == 0 else H), H, f"WT{l}")
                  for l in range(4)]
            Wn = {l: load_bf(dWn[l], H, H, f"Wn{l}") for l in (1, 2, 3)}
            W0n = load_bf(dW0n, H, IN, "W0n")
            W0q = load_bf(dW0q, H, ND, "W0q")
            bs = []
            for l in range(4):
                ps_ = []
                for ki in range(KT):
                    t = wp.tile([128, 1], F32, tag=f"b{l}_{ki}")
                    nc.sync.dma_start(t[:], dbs[l][ki * 128:(ki + 1) * 128, :])
                    ps_.append(t)
                bs.append(ps_)
            w4t = []
            for ki in range(KT):
                t = wp.tile([128, 1], F32, tag=f"w4_{ki}")
                nc.sync.dma_start(t[:], dw4[ki * 128:(ki + 1) * 128, :])
                w4t.append(t)

            W0qT = []
            for ki in range(KT):
                t = wp.tile([128, ND, T], BF, tag=f"W0qT_{ki}")
                nc.vector.tensor_scalar_mul(
                    t[:], W0q[ki][:].unsqueeze(2).broadcast_to((128, ND, T)), 1.0)
                W0qT.append(t)

            # inputs: X^T (bf16) for the forward pass, qdot tile for Coriolis
            XT = actp.tile([IN, N], BF)
            for g in range(NG):
                xt = xp.tile([128, IN], F32, tag="xt")
                nc.sync.dma_start(xt[:, 0:ND], dq[g * 128:(g + 1) * 128, :])
                nc.sync.dma_start(xt[:, ND:], dqd[g * 128:(g + 1) * 128, :])
                pt = psB.tile([128, 1024], F32, tag="ps", name="ptx")
                nc.tensor.transpose(pt[0:IN, 0:128], xt[:], ident[:])
                nc.vector.tensor_scalar_mul(XT[:, g * 128:(g + 1) * 128],
                                            pt[0:IN, 0:128], 1.0)
            Xqs = tlp.tile([128, NG, ND], F32)
            nc.sync.dma_start(
                Xqs[:], dqd[:].rearrange("(g p) r -> p g r", p=128))
            Xqd = tlp.tile([128, NG, ND], BF)
            nc.vector.tensor_scalar_mul(Xqd[:], Xqs[:], 1.0)

            # ---------------- phase 1 (per half h) ---------------------------
            # A = W z + b (f32 psum); P=exp(-|A|); L=ln(1+P); S=exp(min(A,0)-L);
            # Z = relu(A)+L.
            Sbf = [[[None] * KT for _ in range(4)] for _ in range(2)]
            Zbf = [[[None] * KT for _ in range(3)] for _ in range(2)]
            c4h = [[None] * KT for _ in range(2)]
            F3h = [[None] * KT for _ in range(2)]
            F2h = [[None] * KT for _ in range(2)]
            E1h = [[None] * KT for _ in range(2)]
            Rt = tlp.tile([128, ND, NG], F32)

            def fwd_unit(h, l, ot, parts):
                ps = psB.tile([128, 1024], F32, tag="ps", name="fps")
                nk = len(parts)
                for ki in range(nk):
                    nc.tensor.matmul(ps[:, 0:NH],
                                     WT[l][ki][:, ot * 128:(ot + 1) * 128],
                                     parts[ki], start=(ki == 0), stop=(ki == nk - 1))
                A = ph1.tile([128, NH], F32, tag=f"A{ot}")
                nc.scalar.activation(A[:], ps[:, 0:NH], AF.Identity, bias=bs[l][ot][:])
                ab = ph1.tile([128, NH], F32, tag=f"t1{ot}")
                nc.scalar.activation(ab[:], A[:], AF.Abs)
                P = ph1.tile([128, NH], F32, tag=f"t2{ot}")
                nc.scalar.activation(P[:], ab[:], AF.Exp, scale=-1.0)
                L = ph1.tile([128, NH], F32, tag=f"t1{ot}")
                nc.scalar.activation(L[:], P[:], AF.Ln, bias=1.0)
                d = ph1.tile([128, NH], F32, tag=f"t2{ot}")
                nc.vector.scalar_tensor_tensor(d[:], A[:], 0.0, L[:],
                                               ALU.min, ALU.subtract)
                S = actp.tile([128, NH], BF, tag=f"S{l}_{ot}_{h}", name="S")
                nc.scalar.activation(S[:], d[:], AF.Exp)
                Sbf[h][l][ot] = S
                if l < 3:
                    Z = actp.tile([128, NH], BF, tag=f"Z{l}_{ot}_{h}", name="Z")
                    nc.vector.scalar_tensor_tensor(Z[:], A[:], 0.0, L[:],
                                                   ALU.max, ALU.add)
                    Zbf[h][l][ot] = Z

            def bwd_unit(h, l, Dup, want_F):
                Ds, Fs = [], []
                for ot in range(KT):
                    ps = psB.tile([128, 1024], F32, tag="ps", name="bps")
                    for ki in range(KT):
                        nc.tensor.matmul(ps[:, 0:NH],
                                         Wn[l][ki][:, ot * 128:(ot + 1) * 128],
                                         Dup[ki][:],
                                         start=(ki == 0), stop=(ki == KT - 1))
                    U = ph1.tile([128, NH], BF, tag=f"U{ot}")
                    nc.scalar.activation(U[:], ps[:, 0:NH], AF.Copy)
                    D = actp.tile([128, NH], BF, tag=f"D{l}_{ot}_{h}", name="D")
                    nc.vector.tensor_mul(D[:], Sbf[h][l - 1][ot][:], U[:])
                    Ds.append(D)
                    if want_F:
                        F = actp.tile([128, NH], BF, tag=f"F{l}_{ot}_{h}", name="F")
                        nc.vector.scalar_tensor_tensor(F[:], D[:], -1.0, U[:],
                                                       ALU.mult, ALU.add)
                        Fs.append(F)
                return Ds, Fs

            def phase1_half(h):
                hs = slice(h * NH, (h + 1) * NH)
                for ot in range(KT):
                    fwd_unit(h, 0, ot, [XT[:, hs]])
                for l in range(1, 4):
                    for ot in range(KT):
                        fwd_unit(h, l, ot,
                                 [Zbf[h][l - 1][k][:] for k in range(KT)])
                D4 = []
                for ot in range(KT):
                    Dt = actp.tile([128, NH], BF, tag=f"D4_{ot}_{h}", name="D4")
                    nc.vector.tensor_scalar_mul(Dt[:], Sbf[h][3][ot][:], w4t[ot][:])
                    D4.append(Dt)
                    t1 = ph1.tile([128, NH], BF, tag=f"bt{ot}")
                    nc.vector.tensor_mul(t1[:], Dt[:], Sbf[h][3][ot][:])
                    c = actp.tile([128, NH], BF, tag=f"c4_{ot}_{h}", name="c4")
                    nc.vector.scalar_tensor_tensor(c[:], t1[:], -1.0, Dt[:],
                                                   ALU.mult, ALU.add)
                    c4h[h][ot] = c
                D3, F3 = bwd_unit(h, 3, D4, True)
                D2, F2 = bwd_unit(h, 2, D3, True)
                D1, _ = bwd_unit(h, 1, D2, False)
                F3h[h], F2h[h] = F3, F2
                for ot in range(KT):
                    t1 = ph1.tile([128, NH], BF, tag=f"bt{ot}")
                    nc.vector.tensor_mul(t1[:], D1[ot][:], Sbf[h][0][ot][:])
                    E = actp.tile([128, NH], BF, tag=f"E1_{ot}_{h}", name="E1")
                    nc.vector.scalar_tensor_tensor(E[:], t1[:], -1.0, D1[ot][:],
                                                   ALU.mult, ALU.add)
                    E1h[h][ot] = E
                Gps = psB.tile([128, 1024], F32, tag="ps", name="gps")
                for ki in range(KT):
                    nc.tensor.matmul(Gps[0:IN, 0:NH], W0n[ki][:], D1[ki][:],
                                     start=(ki == 0), stop=(ki == KT - 1))
                G = actp.tile([ND, NH], F32, tag=f"G_{h}", name="G")
                nc.scalar.activation(G[:], Gps[0:ND, 0:NH], AF.Copy)
                for gl in range(NG // 2):
                    g = h * (NG // 2) + gl
                    ptg = psB.tile([128, 1024], F32, tag="ps", name="ptg")
                    nc.tensor.transpose(ptg[:, 0:ND],
                                        G[:, gl * 128:(gl + 1) * 128],
                                        ident[0:ND, 0:ND])
                    nc.vector.tensor_scalar_mul(Rt[:, :, g], ptg[:, 0:ND], 100.0)

            # ---------------- tangent blocks (pairs) -------------------------
            Ubf = tlp.tile([128, NG, ND, 32], BF)
            Hcs = []
            for par in range(2):
                hct = hcp.tile([32, ND, 128], BF, tag=f"Hc{par}", name=f"Hc{par}")
                nc.gpsimd.memset(hct[:], 0.0)
                Hcs.append(hct)

            def flat(x):
                return x[:].rearrange("p d t -> p (d t)")

            def tangent_pair(g):
                pb = 2 * g
                bs2 = (pb, pb + 1)
                h = g // (NG // 2)
                sls = {b: slice((b - h * 8) * T, (b - h * 8 + 1) * T) for b in bs2}

                def bca(Sten, ot, b):
                    return Sten[ot][:, sls[b]].unsqueeze(1).broadcast_to(
                        (128, ND, T))

                def fmul(out_t, sten, ot, in_t, b):
                    nc.vector.tensor_tensor(flat(out_t), bca(sten, ot, b),
                                            flat(in_t), ALU.mult)

                def mk(tag, b):
                    pool = zdp if tag.startswith(("Zd2", "Zd3")) else tgp
                    return pool.tile([128, ND, T], BF, tag=f"{tag}_{b % 2}",
                                     name=tag)

                def mm_step(movs, lhs_parts, ot):
                    # movs: list of (tile, ki); accumulate all into one psum
                    ps = psB.tile([128, 1024], F32, tag="ps", name="tps")
                    nsrc = len(movs)
                    for si, (mvt, ki) in enumerate(movs):
                        mv = flat(mvt)
                        for c0 in range(0, FD, 512):
                            ce = min(FD, c0 + 512)
                            nc.tensor.matmul(
                                ps[:, c0:ce],
                                lhs_parts[ki][:, ot * 128:(ot + 1) * 128],
                                mv[:, c0:ce],
                                start=(si == 0), stop=(si == nsrc - 1))
                    return ps

                def evac_step(ps, ot, b):
                    ev = evp.tile([128, ND, T], BF, tag=f"ev{ot}_{b % 2}",
                                  name="ev")
                    nc.scalar.activation(flat(ev), ps[:, 0:FD], AF.Copy)
                    return ev

                st = {b: {} for b in bs2}

                for b in bs2:
                    for ot in range(KT):
                        z = mk(f"Zd1_{ot}", b)
                        fmul(z, Sbf[h][0], ot, W0qT[ot], b)
                        st[b][f"Zd1_{ot}"] = z

                # forward tangent sweep
                for (src, dst, Wl, Sl) in (("Zd1", "Zd2", WT[1], Sbf[h][1]),
                                           ("Zd2", "Zd3", WT[2], Sbf[h][2]),
                                           ("Zd3", "Dd4", WT[3], c4h[h])):
                    pss = {}
                    for b in bs2:
                        for ot in range(KT):
                            pss[(b, ot)] = mm_step(
                                [(st[b][f"{src}_{k}"], k) for k in range(KT)],
                                Wl, ot)
                    evs = {}
                    for b in bs2:
                        for ot in range(KT):
                            evs[(b, ot)] = evac_step(pss[(b, ot)], ot, b)
                    for b in bs2:
                        for ot in range(KT):
                            z = mk(f"{dst}_{ot}", b)
                            fmul(z, Sl, ot, evs[(b, ot)], b)
                            st[b][f"{dst}_{ot}"] = z

                # backward tangent sweep; Dd_l = ta + tb is folded into the
                # next matmul's PSUM accumulation (both ta and tb are moving
                # operands), so no explicit add op exists.
                for (lsrc, l, Sl, Fl, zsrc) in (
                        ("D4", 3, Sbf[h][2], F3h[h], "Zd3"),
                        ("t3", 2, Sbf[h][1], F2h[h], "Zd2"),
                        ("t2", 1, Sbf[h][0], E1h[h], None)):
                    pss = {}
                    for b in bs2:
                        for ot in range(KT):
                            if lsrc == "D4":
                                movs = [(st[b][f"Dd4_{k}"], k) for k in range(KT)]
                            else:
                                movs = []
                                for k in range(KT):
                                    movs.append((st[b][f"tat{lsrc[1]}_{k}"], k))
                                    movs.append((st[b][f"tbt{lsrc[1]}_{k}"], k))
                            pss[(b, ot)] = mm_step(movs, Wn[l], ot)
                    evs = {}
                    for b in bs2:
                        for ot in range(KT):
                            evs[(b, ot)] = evac_step(pss[(b, ot)], ot, b)
                    for b in bs2:
                        for ot in range(KT):
                            tb = mk(f"tbt{l}_{ot}", b)
                            fmul(tb, Sl, ot, evs[(b, ot)], b)
                            ta = mk(f"tat{l}_{ot}", b)
                            if zsrc is None:
                                fmul(ta, Fl, ot, W0qT[ot], b)
                            else:
                                fmul(ta, Fl, ot, st[b][f"{zsrc}_{ot}"], b)
                            st[b][f"tat{l}_{ot}"] = ta
                            st[b][f"tbt{l}_{ot}"] = tb

                # H = W0^T (ta1 + tb1)
                Hc = Hcs[g % 2]
                psHs = {}
                for b in bs2:
                    movs = []
                    for k in range(KT):
                        movs.append((st[b][f"tat1_{k}"], k))
                        movs.append((st[b][f"tbt1_{k}"], k))
                    ps = psB.tile([128, 1024], F32, tag="ps", name="psH")
                    for si, (mvt, ki) in enumerate(movs):
                        mv = flat(mvt)
                        for c0 in range(0, FD, 512):
                            ce = min(FD, c0 + 512)
                            nc.tensor.matmul(ps[0:IN, c0:ce], W0n[ki][:],
                                             mv[:, c0:ce],
                                             start=(si == 0), stop=(si == 3))
                    psHs[b] = ps
                for b in bs2:
                    off = (b % 2) * T
                    hsl = Hc[0:IN, :, off:off + T]
                    psr = psHs[b][0:IN, 0:FD].rearrange("p (d t) -> p d t", d=ND)
                    if b % 2 == 0:
                        nc.scalar.activation(hsl, psr, AF.Copy)
                    else:
                        nc.vector.tensor_scalar_mul(hsl, psr, 1.0)
                nc.sync.dma_start(dhsc[g], Hc[:].rearrange("p d t -> p (d t)"))
                nc.sync.dma_start_transpose(out=Ubf[:, g, :, :], in_=dhsc[g])

            # ---- emission: h0 phase-1, pair 0, h1 phase-1, pairs 1..7 -------
            phase1_half(0)
            tangent_pair(0)
            phase1_half(1)
            for g in range(1, NG):
                tangent_pair(g)

            # ---------------- tail: Coriolis, rhs, Jacobi solve --------------
            ct = tlp.tile([128, NG, ND, ND], BF)
            nc.vector.tensor_tensor(
                ct[:], Ubf[:, :, :, 0:ND],
                Xqd[:].unsqueeze(2).broadcast_to((128, NG, ND, ND)), ALU.mult)
            corT = tlp.tile([128, NG, ND], F32)
            nc.vector.tensor_reduce(corT[:].unsqueeze(3), ct[:],
                                    axis=mybir.AxisListType.X, op=ALU.add)
            r100 = tlp.tile([128, ND, NG], F32)
            nc.vector.scalar_tensor_tensor(
                r100[:], corT[:].rearrange("p g i -> p i g"), -100.0, Rt[:],
                ALU.mult, ALU.add)
            Mbf = tlp.tile([128, ND, NG, ND], BF)
            nc.scalar.activation(
                Mbf[:], Ubf[:, :, :, 12:24].rearrange("p g j i -> p i g j"),
                AF.Copy)
            xs = []
            for it in range(2):
                xs.append(tlp.tile([128, NG, ND], BF, tag=f"x{it}", name=f"x{it}"))
            nc.vector.tensor_scalar_mul(
                xs[0][:], r100[:].rearrange("p i g -> p g i"), 1.0)
            xf = tlp.tile([128, NG, ND], F32)
            for it in range(JAC_ITERS):
                xin = xs[it % 2]
                tmp = tlp.tile([128, ND, NG, ND], BF, tag=f"jt{it % 2}", name="jt")
                nc.vector.tensor_tensor(
                    tmp[:], Mbf[:],
                    xin[:].unsqueeze(1).broadcast_to((128, ND, NG, ND)), ALU.mult)
                y = tlp.tile([128, ND, NG], F32, tag=f"y{it % 2}", name="y")
                nc.vector.tensor_reduce(y[:].unsqueeze(3), tmp[:],
                                        axis=mybir.AxisListType.X, op=ALU.add)
                if it < JAC_ITERS - 1:
                    nc.vector.scalar_tensor_tensor(
                        xs[(it + 1) % 2][:], y[:].rearrange("p i g -> p g i"),
                        -100.0, r100[:].rearrange("p i g -> p g i"),
                        ALU.mult, ALU.add)
                else:
                    nc.vector.scalar_tensor_tensor(
                        xf[:], y[:].rearrange("p i g -> p g i"), -100.0,
                        r100[:].rearrange("p i g -> p g i"), ALU.mult, ALU.add)

            nc.sync.dma_start(
                dout[:].rearrange("(g p) i -> p g i", p=128), xf[:])

    nc.compile()
    return nc


def kernel(**inputs):
    q = np.ascontiguousarray(inputs["q"], dtype=np.float32)
    qdot = np.ascontiguousarray(inputs["qdot"], dtype=np.float32)
    if "nc" not in _cache:
        _cache["nc"] = build_kernel()
    nc = _cache["nc"]

    def b(x):
        return np.ascontiguousarray(x).astype(np.float32).astype(bf16)

    W0 = np.asarray(inputs["W0"], dtype=np.float32)
    base = {
        "WT0": b(W0.T),
        "WT1": b(np.asarray(inputs["W1"]).T),
        "WT2": b(np.asarray(inputs["W2"]).T),
        "WT3": b(np.asarray(inputs["W3"]).T),
        "Wn1": b(inputs["W1"]),
        "Wn2": b(inputs["W2"]),
        "Wn3": b(inputs["W3"]),
        "W0n": b(W0),
        "W0q": b(W0[:, ND:]),
        "b0": inputs["b0"].reshape(H, 1).astype(np.float32),
        "b1": inputs["b1"].reshape(H, 1).astype(np.float32),
        "b2": inputs["b2"].reshape(H, 1).astype(np.float32),
        "b3": inputs["b3"].reshape(H, 1).astype(np.float32),
        "w4": np.asarray(inputs["W4"]).reshape(H, 1).astype(np.float32),
        "ident": np.eye(128, dtype=np.float32),
    }
    in_maps = []
    for c in range(NC):
        m = dict(base)
        m["q"] = q[c * N:(c + 1) * N]
        m["qdot"] = qdot[c * N:(c + 1) * N]
        in_maps.append(m)
    res = run_bass_kernel_spmd(nc, in_maps, core_ids=list(range(NC)))
    _cache["res"] = res
    out = np.concatenate([res.results[c]["qdd"] for c in range(NC)], axis=0)
    return out.astype(np.float32)
